# revision 1
# baseline (speedup 1.0000x reference)
"""Trainium2 Bass kernel for a bidirectional cross-attention block.

Reference computation (per batch b):
  t = LN(text[b]);  v = LN(vision[b])
  text_out[b]   = softmax((t@Wq1.T+bq1) (v@Wk2.T+bk2)^T / 8) (v@Wv2.T+bv2) @ Wo1.T + bo1
  vision_out[b] = softmax((v@Wq2.T+bq2) (t@Wk1.T+bk1)^T / 8) (t@Wv1.T+bv1) @ Wo2.T + bo2
  (12 heads of dk=64; D=768, N=2048)

Sharding over 8 cores: (batch b in {0,1}) x (path in {text-q, vision-q}) x
(head-half in {heads 0-5, heads 6-11}).  Each core computes a [2048, 768]
partial of one output (its 6 heads pushed through the output projection);
the host sums the two head-half partials and adds the output bias.

Device kernel (per core) highlights:
  - Host passes x TRANSPOSED (feature-major, bf16) so every matmul contracts
    along partitions; activations are never transposed on device.
  - LN scale/shift folded into the projection weights host-side (exact).
    LN mean/rstd computed on device:  Q = r*(x@W) - (r*mu) x colsum(W),
    applied as two DVE passes per projection tile (colsum identity).
  - K-side biases are row-constant in the scores -> dropped (softmax
    invariant).  The Q bias enters through a per-key correction t_k =
    bq . K_k, computed with tiny matmuls and folded into the exp() bias.
  - exp on ACT with fused scale 1/8 + bias; no max-subtraction (scores are
    bounded by ~6 for LN'd inputs).
  - Row-sums of exp(S) obtained by appending a ones-column to V in the
    P@V matmul; normalization applied after P@V (linearity).
"""

import math
import os
import sys
from contextlib import ExitStack

import numpy as np

for _p in ("/opt/trn_rl_repo", os.path.expanduser("~/.axon_site/_ro/trn_rl_repo")):
    if os.path.isdir(_p) and _p not in sys.path:
        sys.path.insert(0, _p)

import ml_dtypes  # noqa: E402

import concourse.bass as bass  # noqa: E402
import concourse.bacc as bacc  # noqa: E402
import concourse.tile as tile  # noqa: E402
from concourse import mybir  # noqa: E402
from concourse.bass_utils import run_bass_kernel_spmd  # noqa: E402

BF16 = np.dtype(np.float16)  # fp16: same PE rate as bf16, 8x mantissa

NSEQ = 2048
D = 768
HEADS = 12
DK = 64
HPC = 6            # heads per core
F = HPC * DK       # 384 features per core
KT = D // 128      # 6 contraction tiles
FB = F // 128      # 3 feature blocks
NB4 = NSEQ // 512  # 4 seq blocks of 512
NB16 = NSEQ // 128  # 16 seq blocks of 128
EPS = 1e-5
SCALE = DK ** -0.5  # 0.125

_AF = None  # mybir.ActivationFunctionType alias, set in _build


def _bcast_ap(ap, p):
    """[1, ...] SBUF/DRAM AP -> partition-broadcast [p, ...] AP (stride 0)."""
    return bass.AP(tensor=ap.tensor, offset=ap.offset, ap=[[0, p]] + list(ap.ap[1:]))


def _emit(ctx, tc, io):
    nc = tc.nc
    f32 = mybir.dt.float32
    bf16 = mybir.dt.float16
    AF = mybir.ActivationFunctionType
    OP = mybir.AluOpType

    xqT, xkvT, xkv_sm = io["xqT"], io["xkvT"], io["xkv_sm"]
    wq, wk, wv, wo = io["wq"], io["wk"], io["wv"], io["wo"]
    cnq, cnk, cnv, bvr, bqc = io["cnq"], io["cnk"], io["cnv"], io["bvr"], io["bqc"]
    out = io["out"]

    # ---- pools that live for the whole kernel ----
    const = ctx.enter_context(tc.tile_pool(name="const", bufs=1))
    qkv_pool = ctx.enter_context(tc.tile_pool(name="qkv", bufs=1))
    wo_pool = ctx.enter_context(tc.tile_pool(name="wo", bufs=1))
    tpool = ctx.enter_context(tc.tile_pool(name="tsb", bufs=1))
    apool = ctx.enter_context(tc.tile_pool(name="afm", bufs=1))

    ones = const.tile([128, 1], bf16)
    nc.vector.memset(ones, 1.0)
    eps1 = const.tile([1, 1], f32)
    nc.vector.memset(eps1, EPS)
    eps128 = const.tile([128, 1], f32)
    nc.vector.memset(eps128, EPS)
    cnq_sb = const.tile([128, FB], f32)
    nc.sync.dma_start(out=cnq_sb, in_=cnq)
    cnk_sb = const.tile([128, FB], f32)
    nc.sync.dma_start(out=cnk_sb, in_=cnk)
    bqc_sb = const.tile([128, FB], bf16)
    nc.sync.dma_start(out=bqc_sb, in_=bqc)
    cnv_bc = const.tile([128, F], f32)
    nc.gpsimd.dma_start(out=cnv_bc, in_=_bcast_ap(cnv[None, :], 128))
    bv_bc = const.tile([128, F], f32)
    nc.gpsimd.dma_start(out=bv_bc, in_=_bcast_ap(bvr[None, :], 128))

    qt_sb = qkv_pool.tile([128, FB, NSEQ], bf16)   # Q^T feature-major
    kt_sb = qkv_pool.tile([128, FB, NSEQ], bf16)   # K^T feature-major
    v_sb = qkv_pool.tile([128, NB16, HPC, DK + 1], bf16)  # V seq-major + ones col
    a_sb = apool.tile([128, FB, NSEQ], bf16)       # attention out, feature-major

    wo_sb = wo_pool.tile([128, FB, D], bf16)
    for f3 in range(FB):
        nc.sync.dma_start(out=wo_sb[:, f3, :], in_=wo[f3])

    t_sb = tpool.tile([128, HPC, NB16], f32)       # per-key exp-bias (q-bias fold)

    # ================= phase A: load x, stats, projections =================
    with ExitStack() as pre:
        xpool = pre.enter_context(tc.tile_pool(name="xt", bufs=1))
        wpool = pre.enter_context(tc.tile_pool(name="wqkv", bufs=1))
        spool = pre.enter_context(tc.tile_pool(name="stats", bufs=1))
        scr = pre.enter_context(tc.tile_pool(name="scratch", bufs=2))
        sqpool = pre.enter_context(tc.tile_pool(name="sq", bufs=2))
        bcpool = pre.enter_context(tc.tile_pool(name="bc", bufs=1))
        ppool = pre.enter_context(tc.tile_pool(name="pp", bufs=1))
        xsm_pool = pre.enter_context(tc.tile_pool(name="xsm", bufs=3))
        bnp = pre.enter_context(tc.tile_pool(name="bn", bufs=3))
        upool = pre.enter_context(tc.tile_pool(name="u", bufs=3))
        pst = pre.enter_context(tc.tile_pool(name="pstat", bufs=1, space="PSUM"))
        prj_ps = pre.enter_context(tc.tile_pool(name="prj", bufs=1, space="PSUM"))

        xqT_sb = xpool.tile([128, KT, NSEQ], bf16)
        xkvT_sb = xpool.tile([128, KT, NSEQ], bf16)
        for kt in range(KT):
            nc.sync.dma_start(out=xqT_sb[:, kt, :], in_=xqT[kt * 128:(kt + 1) * 128, :])
            nc.sync.dma_start(out=xkvT_sb[:, kt, :], in_=xkvT[kt * 128:(kt + 1) * 128, :])

        wq_sb = wpool.tile([128, KT, F], bf16)
        wk_sb = wpool.tile([128, KT, F], bf16)
        wv_sb = wpool.tile([128, KT, F], bf16)
        for kt in range(KT):
            nc.sync.dma_start(out=wq_sb[:, kt, :], in_=wq[kt])
            nc.sync.dma_start(out=wk_sb[:, kt, :], in_=wk[kt])
            nc.sync.dma_start(out=wv_sb[:, kt, :], in_=wv[kt])

        # ---- row stats (mean, rstd along features) via ones-matmuls ----
        bcs = {}
        for nm, xsb in (("q", xqT_sb), ("kv", xkvT_sb)):
            mu_bf = spool.tile([1, NSEQ], bf16, name=f"mu_bf_{nm}")
            r_bf = spool.tile([1, NSEQ], bf16, name=f"r_bf_{nm}")
            for nb in range(NB4):
                sl = slice(nb * 512, (nb + 1) * 512)
                sq = sqpool.tile([128, KT, 512], bf16)
                nc.vector.tensor_mul(sq, xsb[:, :, sl], xsb[:, :, sl])
                mu_ps = pst.tile([1, 512], f32, name="mu_ps")
                ms_ps = pst.tile([1, 512], f32, name="ms_ps")
                for kt in range(KT):
                    nc.tensor.matmul(mu_ps, ones, xsb[:, kt, sl],
                                     start=(kt == 0), stop=(kt == KT - 1))
                for kt in range(KT):
                    nc.tensor.matmul(ms_ps, ones, sq[:, kt, :],
                                     start=(kt == 0), stop=(kt == KT - 1))
                mu5 = scr.tile([1, 512], f32, name="mu5")
                var5 = scr.tile([1, 512], f32, name="var5")
                mu25 = scr.tile([1, 512], f32, name="mu25")
                nc.vector.tensor_scalar_mul(mu5, mu_ps, 1.0 / D)
                nc.vector.tensor_scalar_mul(var5, ms_ps, 1.0 / D)
                nc.vector.tensor_mul(mu25, mu5, mu5)
                nc.vector.tensor_sub(var5, var5, mu25)
                # var -> sd -> rstd
                nc.scalar.activation(var5, var5, AF.Sqrt, bias=eps1)
                nc.vector.reciprocal(var5, var5)
                nc.vector.tensor_copy(mu_bf[:, sl], mu5)
                nc.vector.tensor_copy(r_bf[:, sl], var5)
            # broadcast rows across partitions (bf16)
            mu_bc = bcpool.tile([128, NSEQ], bf16, name=f"mu_bc_{nm}")
            r_bc = bcpool.tile([128, NSEQ], bf16, name=f"r_bc_{nm}")
            nc.gpsimd.partition_broadcast(mu_bc, mu_bf)
            nc.gpsimd.partition_broadcast(r_bc, r_bf)
            bcs[nm] = (mu_bc, r_bc)

        # ---- kv per-partition stats (for seq-major V fixup) via bn_stats ----
        mu_pp = ppool.tile([128, NB16], f32)
        r_pp = ppool.tile([128, NB16], f32)
        bn_sub = math.gcd(nc.vector.BN_STATS_FMAX, D)   # 256
        nsub = D // bn_sub
        for o in range(NB16):
            xt = xsm_pool.tile([128, D], bf16)
            nc.sync.dma_start(out=xt, in_=xkv_sm[o * 128:(o + 1) * 128, :])
            stats = bnp.tile([128, nsub, nc.vector.BN_STATS_DIM], f32)
            xr = xt.rearrange("p (s d) -> p s d", s=nsub)
            for si in range(nsub):
                nc.vector.bn_stats(out=stats[:, si, :], in_=xr[:, si, :])
            mv = bnp.tile([128, nc.vector.BN_AGGR_DIM], f32)
            nc.vector.bn_aggr(out=mv, in_=stats)
            nc.gpsimd.tensor_copy(out=mu_pp[:, o:o + 1], in_=mv[:, 0:1])
            nc.gpsimd.tensor_copy(out=r_pp[:, o:o + 1], in_=mv[:, 1:2])
        nc.scalar.activation(r_pp, r_pp, AF.Sqrt, bias=eps128)
        nc.vector.reciprocal(r_pp, r_pp)

        # ---- Q^T / K^T projections (feature-major) + LN fixup ----
        for xsb, wsb, cn_sb, dst, nm in (
            (xqT_sb, wq_sb, cnq_sb, qt_sb, "q"),
            (xkvT_sb, wk_sb, cnk_sb, kt_sb, "kv"),
        ):
            mu_bc, r_bc = bcs[nm]
            for fb in range(FB):
                pss = [prj_ps.tile([128, 512], f32, name=f"prj{i}") for i in range(NB4)]
                for kt in range(KT):
                    for nb in range(NB4):
                        nc.tensor.matmul(
                            pss[nb],
                            wsb[:, kt, fb * 128:(fb + 1) * 128],
                            xsb[:, kt, nb * 512:(nb + 1) * 512],
                            start=(kt == 0), stop=(kt == KT - 1))
                for nb in range(NB4):
                    sl = slice(nb * 512, (nb + 1) * 512)
                    u = upool.tile([128, 512], f32, name="u")
                    # u = raw - colsum * mu   (cn_sb holds -colsum)
                    nc.vector.scalar_tensor_tensor(
                        out=u, in0=mu_bc[:, sl], scalar=cn_sb[:, fb:fb + 1],
                        in1=pss[nb], op0=OP.mult, op1=OP.add)
                    nc.vector.tensor_mul(dst[:, fb, sl], u, r_bc[:, sl])

        # ---- V projection (seq-major) + LN fixup + bias ----
        for o in range(NB16):
            ps = prj_ps.tile([128, F], f32, name="vps")
            for kt in range(KT):
                nc.tensor.matmul(ps, xkvT_sb[:, kt, o * 128:(o + 1) * 128],
                                 wv_sb[:, kt, :], start=(kt == 0), stop=(kt == KT - 1))
            uv = upool.tile([128, F], f32, name="uv")
            nc.vector.scalar_tensor_tensor(
                out=uv, in0=cnv_bc, scalar=mu_pp[:, o:o + 1], in1=ps,
                op0=OP.mult, op1=OP.add)
            nc.vector.scalar_tensor_tensor(
                out=v_sb[:, o, :, 0:DK],
                in0=uv.rearrange("p (h d) -> p h d", h=HPC),
                scalar=r_pp[:, o:o + 1],
                in1=bv_bc.rearrange("p (h d) -> p h d", h=HPC),
                op0=OP.mult, op1=OP.add)
        nc.vector.memset(v_sb[:, :, :, DK:DK + 1], 1.0)

        # per-key exp bias t_k = SCALE*(bq . K_k) for all heads (frees a
        # PSUM bank in phase B so score tiles can be 1024 wide)
        for h in range(HPC):
            fb, half = h // 2, (h % 2) * 64
            tps = pst.tile([128, NB16], f32, name="tps")
            for kb in range(NB16):
                nc.tensor.matmul(tps[:, kb:kb + 1],
                                 kt_sb[half:half + 64, fb, kb * 128:(kb + 1) * 128],
                                 bqc_sb[half:half + 64, fb:fb + 1],
                                 start=True, stop=True)
            nc.vector.tensor_scalar_mul(t_sb[:, h, :], tps, SCALE)

    # ================= phase B: attention =================
    attn = ctx.enter_context(ExitStack())
    att_ps = attn.enter_context(tc.tile_pool(name="att", bufs=2, space="PSUM"))
    o_ps_pool = attn.enter_context(tc.tile_pool(name="ops", bufs=1, space="PSUM"))
    ptpool = attn.enter_context(tc.tile_pool(name="pt", bufs=4))
    rspool = attn.enter_context(tc.tile_pool(name="rs", bufs=6))

    AFexp = AF.Exp
    for h in range(HPC):
        fb, half = h // 2, (h % 2) * 64
        opss = [o_ps_pool.tile([DK + 1, 512], f32, name=f"o{qb}") for qb in range(NB4)]
        for kb in range(NB16):
            ksl = kt_sb[half:half + 64, fb, kb * 128:(kb + 1) * 128]
            pts = []
            for t in range(2):
                sps = att_ps.tile([128, 1024], f32, name="sps")
                for g in range(2):
                    qb = 2 * t + g
                    nc.tensor.matmul(sps[:, g * 512:(g + 1) * 512], ksl,
                                     qt_sb[half:half + 64, fb, qb * 512:(qb + 1) * 512],
                                     start=True, stop=True)
                pt = ptpool.tile([128, 1024], bf16, name="pt")
                nc.scalar.activation(pt, sps, AFexp,
                                     bias=t_sb[:, h, kb:kb + 1], scale=SCALE)
                pts.append(pt)
            vsl = v_sb[:, kb, h, :]   # [128, 65]
            for qb in range(NB4):
                nc.tensor.matmul(opss[qb], vsl,
                                 pts[qb // 2][:, (qb % 2) * 512:(qb % 2 + 1) * 512],
                                 start=(kb == 0), stop=(kb == NB16 - 1))
        for qb in range(NB4):
            rs_row = rspool.tile([1, 512], f32, name="rsrow")
            nc.vector.reciprocal(rs_row, opss[qb][DK:DK + 1, :])
            rs_bc = rspool.tile([64, 512], f32, name="rsbc")
            nc.gpsimd.partition_broadcast(rs_bc, rs_row)
            nc.vector.tensor_mul(
                a_sb[half:half + 64, fb, qb * 512:(qb + 1) * 512],
                opss[qb][0:DK, :], rs_bc)

    # ================= phase C: output projection =================
    attn.close()
    op_ps = ctx.enter_context(tc.tile_pool(name="oprj", bufs=2, space="PSUM"))
    outpool = ctx.enter_context(tc.tile_pool(name="outsb", bufs=3))
    for mb in range(NB16):
        pss = [op_ps.tile([128, 384], f32, name=f"op{j}") for j in range(2)]
        for kt3 in range(FB):
            asl = a_sb[:, kt3, mb * 128:(mb + 1) * 128]
            for j in range(2):
                nc.tensor.matmul(pss[j], asl, wo_sb[:, kt3, j * 384:(j + 1) * 384],
                                 start=(kt3 == 0), stop=(kt3 == FB - 1))
        osb = outpool.tile([128, D], f32)
        for j in range(2):
            nc.vector.tensor_copy(osb[:, j * 384:(j + 1) * 384], pss[j])
        nc.sync.dma_start(out=out[mb * 128:(mb + 1) * 128, :], in_=osb)


def _build():
    nc = bacc.Bacc("TRN2", target_bir_lowering=False, debug=False, num_devices=8)
    dt = mybir.dt

    def din(name, shape, dtype):
        return nc.dram_tensor(name, list(shape), dtype, kind="ExternalInput").ap()

    io = {
        "xqT": din("xqT", (D, NSEQ), dt.float16),
        "xkvT": din("xkvT", (D, NSEQ), dt.float16),
        "xkv_sm": din("xkv_sm", (NSEQ, D), dt.float16),
        "wq": din("wq", (KT, 128, F), dt.float16),
        "wk": din("wk", (KT, 128, F), dt.float16),
        "wv": din("wv", (KT, 128, F), dt.float16),
        "wo": din("wo", (FB, 128, D), dt.float16),
        "cnq": din("cnq", (128, FB), dt.float32),
        "cnk": din("cnk", (128, FB), dt.float32),
        "cnv": din("cnv", (F,), dt.float32),
        "bvr": din("bvr", (F,), dt.float32),
        "bqc": din("bqc", (128, FB), dt.float16),
        "out": nc.dram_tensor("out", [NSEQ, D], dt.float32, kind="ExternalOutput").ap(),
    }

    with tile.TileContext(nc) as tc:
        with ExitStack() as ctx:
            _emit(ctx, tc, io)
    nc.compile()
    return nc


_CACHE = {}


def _get_nc():
    if "nc" not in _CACHE:
        _CACHE["nc"] = _build()
    return _CACHE["nc"]


def _prep(inputs):
    g = lambda k: np.asarray(inputs[k], dtype=np.float32)
    text, vision = g("text"), g("vision")
    ln1_w, ln1_b, ln2_w, ln2_b = g("ln1_w"), g("ln1_b"), g("ln2_w"), g("ln2_b")
    W = {nm: g("W" + nm) for nm in ("q1", "k1", "v1", "q2", "k2", "v2", "o1", "o2")}
    B = {nm: g("b" + nm) for nm in ("q1", "k1", "v1", "q2", "k2", "v2", "o1", "o2")}

    maps = [None] * 8
    for b in (0, 1):
        for path in (0, 1):
            if path == 0:
                xq, xkv = text[b], vision[b]
                lnqw, lnqb, lnkw, lnkb = ln1_w, ln1_b, ln2_w, ln2_b
                Wq, bq, Wk, Wv, bv, Wo = W["q1"], B["q1"], W["k2"], W["v2"], B["v2"], W["o1"]
            else:
                xq, xkv = vision[b], text[b]
                lnqw, lnqb, lnkw, lnkb = ln2_w, ln2_b, ln1_w, ln1_b
                Wq, bq, Wk, Wv, bv, Wo = W["q2"], B["q2"], W["k1"], W["v1"], B["v1"], W["o2"]
            xqT = np.ascontiguousarray(xq.T).astype(BF16)
            xkvT = np.ascontiguousarray(xkv.T).astype(BF16)
            xkv_sm = xkv.astype(BF16)
            for s in (0, 1):
                rows = slice(s * F, (s + 1) * F)
                WqT = np.ascontiguousarray((lnqw[:, None] * Wq[rows].T)).astype(BF16)
                WkT = np.ascontiguousarray((lnkw[:, None] * Wk[rows].T)).astype(BF16)
                WvT = np.ascontiguousarray((lnkw[:, None] * Wv[rows].T)).astype(BF16)
                cq = -WqT.astype(np.float32).sum(0)   # [F]
                ck = -WkT.astype(np.float32).sum(0)
                cv = -WvT.astype(np.float32).sum(0)
                bq_eff = (bq[rows] + lnqb @ Wq[rows].T).astype(np.float32)
                bv_eff = (bv[rows] + lnkb @ Wv[rows].T).astype(np.float32)
                WoT = np.ascontiguousarray(Wo[:, rows].T).astype(BF16)  # [F, D]
                maps[b * 4 + path * 2 + s] = {
                    "xqT": xqT, "xkvT": xkvT, "xkv_sm": xkv_sm,
                    "wq": WqT.reshape(KT, 128, F),
                    "wk": WkT.reshape(KT, 128, F),
                    "wv": WvT.reshape(KT, 128, F),
                    "wo": WoT.reshape(FB, 128, D),
                    "cnq": np.ascontiguousarray(cq.reshape(FB, 128).T),
                    "cnk": np.ascontiguousarray(ck.reshape(FB, 128).T),
                    "cnv": cv,
                    "bvr": bv_eff,
                    "bqc": np.ascontiguousarray(bq_eff.reshape(FB, 128).T).astype(BF16),
                }
    meta = (B["o1"], B["o2"])
    return maps, meta


def _unshard(results, meta):
    bo1, bo2 = meta
    text_out = np.empty((2, NSEQ, D), np.float32)
    vision_out = np.empty((2, NSEQ, D), np.float32)
    for b in (0, 1):
        text_out[b] = results[b * 4 + 0]["out"] + results[b * 4 + 1]["out"] + bo1
        vision_out[b] = results[b * 4 + 2]["out"] + results[b * 4 + 3]["out"] + bo2
    return (text_out, vision_out)


def run_raw(inputs, **kw):
    """Run and return the BassKernelResults (for profiling from test.py)."""
    nc = _get_nc()
    in_maps, meta = _prep(inputs)
    res = run_bass_kernel_spmd(nc, in_maps, core_ids=list(range(8)), **kw)
    return res, meta


def kernel(**inputs):
    res, meta = run_raw(inputs)
    return _unshard(res.results, meta)



# revision 10
# speedup vs baseline: 1.0132x; 1.0132x over previous
"""Trainium2 Bass kernel for a bidirectional cross-attention block.

Reference computation (per batch b):
  t = LN(text[b]);  v = LN(vision[b])
  text_out[b]   = softmax((t@Wq1.T+bq1) (v@Wk2.T+bk2)^T / 8) (v@Wv2.T+bv2) @ Wo1.T + bo1
  vision_out[b] = softmax((v@Wq2.T+bq2) (t@Wk1.T+bk1)^T / 8) (t@Wv1.T+bv1) @ Wo2.T + bo2
  (12 heads of dk=64; D=768, N=2048)

Sharding over 8 cores: (batch b in {0,1}) x (path in {text-q, vision-q}) x
(head-half in {heads 0-5, heads 6-11}).  Each core computes a [2048, 768]
partial of one output (its 6 heads pushed through the output projection);
the host sums the two head-half partials (fp16) and adds the output bias.

Device kernel (per core) highlights:
  - x passed TRANSPOSED (feature-major, fp16); LN weight folded into the
    projection weights host-side.
  - Per-token LN stats via 1-column matmuls (x tile as stationary, ones as
    moving): seq-major sums cost ~1 PE cycle each.  Stats are finished on
    tiny [128,16] tiles, then scattered to row layout by DMA.
  - LN applied as: prescale x by rstd (DVE, fp16 2x), then the mean term is
    a rank-1 PSUM update — one extra matmul row per accumulation group
    (lhsT = -colsum(W), rhs = (mu*rstd) row).  V also folds its bias via a
    ones row.  No per-tile fixup passes.
  - K-side biases dropped (softmax invariant); Q bias enters through a
    per-key correction t_k = bq . K_k folded into the exp bias.
  - exp split between ACT (native Exp) and DVE (custom EXP8 op: deg-2
    minimax parabola of e^{u/64} raised to the 8th power in one 8-stage
    pass, then 3 fp16 squarings -> e^u).  Rebalances the ACT bottleneck.
  - Row-sums of exp(S) via an appended ones-column in V; normalization
    applied after P@V (linearity).
  - Output written fp16 (halves write traffic); host sums partials in f32.
"""

import math
import os
import sys
from contextlib import ExitStack

import numpy as np

for _p in ("/opt/trn_rl_repo", os.path.expanduser("~/.axon_site/_ro/trn_rl_repo")):
    if os.path.isdir(_p) and _p not in sys.path:
        sys.path.insert(0, _p)

import ml_dtypes  # noqa: E402

import concourse.bass as bass  # noqa: E402
import concourse.bacc as bacc  # noqa: E402
import concourse.tile as tile  # noqa: E402
from concourse import mybir  # noqa: E402
from concourse import dve_ops  # noqa: E402
from concourse.dve_ops import DveOp  # noqa: E402
from concourse.dve_spec import Spec, Src0, C0, C1, C2, One, sq  # noqa: E402
from concourse.bass_utils import run_bass_kernel_spmd  # noqa: E402

F16 = np.dtype(np.float16)

NSEQ = 2048
D = 768
HEADS = 12
DK = 64
HPC = 6            # heads per core
F = HPC * DK       # 384 features per core
KT = D // 128      # 6 contraction tiles
FB = F // 128      # 3 feature blocks
NB16 = NSEQ // 128  # 16 seq blocks of 128
EPS = 1e-5
SCALE = DK ** -0.5  # 0.125

# minimax parabola e^v ~= C2*((v+B)^2+1) scaled:  w = A*v + B on |v|<=0.117
EXP_A = 1.0017179402073273
EXP_B = 1.0042971728803987
EXP_C2 = 0.4978589582950239
EXP_C1 = SCALE * EXP_A / 64.0   # multiplies the raw score


def _ref_exp8(in0, in1, c0, c1, c2):
    w = in0.astype(np.float32) * np.float32(c1) + np.asarray(c0, np.float32)
    q = (w * w + np.float32(1.0)) * np.float32(c2)
    q2 = q * q
    q4 = q2 * q2
    return q4 * q4


_w = Src0 * C1 + C0
EXP8_ANT = DveOp(
    "EXP8_ANT",
    Spec(body=sq(sq(sq((sq(_w) + One) * C2))), reference=_ref_exp8),
    subdim=False,
    uops_sha={"v3": "0772b029163394d3"},
)

if EXP8_ANT.name not in dve_ops._SUB_OPCODE_FOR_NAME:
    dve_ops.OPS.append(EXP8_ANT)
    dve_ops.CUSTOM_DVE_SPECS[EXP8_ANT.name] = EXP8_ANT.spec
    dve_ops._SUB_OPCODE_FOR_NAME[EXP8_ANT.name] = max(
        dve_ops._SUB_OPCODE_FOR_NAME.values()) + 1
    try:
        EXP8_ANT.compile("v3")
    except ValueError as e:  # sha drift: re-pin from the error message
        import re
        m = re.search(r"v3: ([0-9a-f]+)", str(e))
        if m:
            EXP8_ANT.uops_sha["v3"] = m.group(1)


def _dve_exp_tile(h, kb, t):
    """Which exp tiles run on DVE (custom op) instead of ACT."""
    return t == 1 and (kb % 5) < 2


def _emit(ctx, tc, io):
    nc = tc.nc
    f32 = mybir.dt.float32
    f16 = mybir.dt.float16
    AF = mybir.ActivationFunctionType

    xqT, xkvT = io["xqT"], io["xkvT"]
    wq, wk, wv, wo = io["wq"], io["wk"], io["wv"], io["wo"]
    cnq, cnk, cbv, bqc = io["cnq"], io["cnk"], io["cbv"], io["bqc"]
    out = io["out"]

    # ---- pools that live for the whole kernel ----
    const = ctx.enter_context(tc.tile_pool(name="const", bufs=1))
    qkv_pool = ctx.enter_context(tc.tile_pool(name="qkv", bufs=1))
    wo_pool = ctx.enter_context(tc.tile_pool(name="wo", bufs=1))
    tpool = ctx.enter_context(tc.tile_pool(name="tsb", bufs=1))
    apool = ctx.enter_context(tc.tile_pool(name="afm", bufs=1))

    ones = const.tile([128, 1], f16)
    nc.vector.memset(ones, 1.0)
    eps1 = const.tile([128, 1], f32)
    nc.vector.memset(eps1, EPS)
    cnq_sb = const.tile([1, FB, 128], f16)
    nc.sync.dma_start(out=cnq_sb, in_=cnq)
    cnk_sb = const.tile([1, FB, 128], f16)
    nc.sync.dma_start(out=cnk_sb, in_=cnk)
    cbv_sb = const.tile([2, F], f16)
    nc.sync.dma_start(out=cbv_sb, in_=cbv)
    bqc_sb = const.tile([128, FB], f16)
    nc.sync.dma_start(out=bqc_sb, in_=bqc)
    # aug rows: vaug p0 = (mu*rstd)_kv, p1 = ones;  mrq p0 = (mu*rstd)_q
    vaug = const.tile([2, NSEQ], f16)
    nc.vector.memset(vaug[1:2, :], 1.0)
    mrq = const.tile([1, NSEQ], f16)
    rrow = const.tile([2, NSEQ], f16)   # p0 = rstd_q row, p1 = rstd_kv row

    qt_sb = qkv_pool.tile([128, FB, NSEQ], f16)   # Q^T feature-major
    kt_sb = qkv_pool.tile([128, FB, NSEQ], f16)   # K^T feature-major
    v_sb = qkv_pool.tile([128, NB16, HPC, DK + 1], f16)  # V seq-major + ones
    a_sb = apool.tile([128, FB, NSEQ], f16)       # attention out, feature-major

    wo_sb = wo_pool.tile([128, FB, D], f16)
    for f3 in range(FB):
        nc.sync.dma_start(out=wo_sb[:, f3, :], in_=wo[f3])

    t_sb = tpool.tile([128, HPC, NB16], f32)   # exp bias for ACT tiles
    t8_sb = tpool.tile([128, HPC, NB16], f32)  # exp bias for DVE tiles

    # ================= phase A: load x, stats, prescale, projections ========
    with ExitStack() as pre:
        xpool = pre.enter_context(tc.tile_pool(name="xt", bufs=1))
        wpool = pre.enter_context(tc.tile_pool(name="wqkv", bufs=1))
        sqpool = pre.enter_context(tc.tile_pool(name="sq", bufs=2))
        scr = pre.enter_context(tc.tile_pool(name="scratch", bufs=2))
        bcpool = pre.enter_context(tc.tile_pool(name="bc", bufs=2))
        pst = pre.enter_context(tc.tile_pool(name="pstat", bufs=1, space="PSUM"))
        prj_ps = pre.enter_context(tc.tile_pool(name="prj", bufs=2, space="PSUM"))
        v_ps = pre.enter_context(tc.tile_pool(name="vprj", bufs=2, space="PSUM"))

        xq_sb = xpool.tile([128, KT, NSEQ], f16)
        xkv_sb = xpool.tile([128, KT, NSEQ], f16)
        for kt in range(KT):
            nc.sync.dma_start(out=xq_sb[:, kt, :], in_=xqT[kt])
            nc.sync.dma_start(out=xkv_sb[:, kt, :], in_=xkvT[kt])

        wq_sb = wpool.tile([128, KT, F], f16)
        wk_sb = wpool.tile([128, KT, F], f16)
        wv_sb = wpool.tile([128, KT, F], f16)
        for kt in range(KT):
            nc.sync.dma_start(out=wq_sb[:, kt, :], in_=wq[kt])
            nc.sync.dma_start(out=wk_sb[:, kt, :], in_=wk[kt])
            nc.sync.dma_start(out=wv_sb[:, kt, :], in_=wv[kt])

        # ---- per-token stats (seq-major) via 1-column matmuls ----
        rbcs = {}
        for idx, (nm, xsb) in enumerate((("q", xq_sb), ("kv", xkv_sb))):
            s1 = pst.tile([128, NB16], f32, name="s1")
            s2 = pst.tile([128, NB16], f32, name="s2")
            # token t maps to (partition t//16, column t%16) in the stat tiles
            for kt in range(KT):
                sqc = sqpool.tile([128, NSEQ], f16, name="sqc")
                nc.vector.tensor_mul(sqc, xsb[:, kt, :], xsb[:, kt, :])
                xr = xsb[:, kt, :].rearrange("p (q s) -> p s q", s=NB16)
                sr = sqc.rearrange("p (q s) -> p s q", s=NB16)
                for tb in range(NB16):
                    nc.tensor.matmul(s1[:, tb:tb + 1], xr[:, tb, :], ones,
                                     start=(kt == 0), stop=(kt == KT - 1))
                    nc.tensor.matmul(s2[:, tb:tb + 1], sr[:, tb, :], ones,
                                     start=(kt == 0), stop=(kt == KT - 1))
            mu = scr.tile([128, NB16], f32, name="mu")
            var = scr.tile([128, NB16], f32, name="var")
            nc.vector.tensor_scalar_mul(mu, s1, 1.0 / D)
            nc.vector.tensor_scalar_mul(var, s2, 1.0 / D)
            mu2 = scr.tile([128, NB16], f32, name="mu2")
            nc.vector.tensor_mul(mu2, mu, mu)
            nc.vector.tensor_sub(var, var, mu2)
            nc.scalar.activation(var, var, AF.Sqrt, bias=eps1)
            nc.vector.reciprocal(var, var)          # var now holds rstd
            nc.vector.tensor_mul(mu2, mu, var)      # mu2 now holds mu*rstd
            rt = scr.tile([128, 2 * NB16], f16, name="rt")
            nc.vector.tensor_copy(rt[:, 0:NB16], var)
            nc.vector.tensor_copy(rt[:, NB16:2 * NB16], mu2)
            # scatter to row layout: token t = p*16 + col, so p-major flatten
            # of rt[:, 0:16] is already linear token order.
            nc.sync.dma_start(out=rrow[idx:idx + 1, :], in_=rt[:, 0:NB16])
            mr_dst = (mrq if nm == "q" else vaug[0:1, :])
            nc.sync.dma_start(out=mr_dst, in_=rt[:, NB16:2 * NB16])
            rbc = bcpool.tile([128, NSEQ], f16, name=f"rbc_{nm}")
            nc.gpsimd.partition_broadcast(rbc, rrow[idx:idx + 1, :])
            rbcs[nm] = rbc

        # ---- prescale x by rstd (token-wise) ----
        for nm, xsb in (("q", xq_sb), ("kv", xkv_sb)):
            rbc = rbcs[nm]
            for kt in range(KT):
                nc.vector.tensor_mul(xsb[:, kt, :], xsb[:, kt, :], rbc)

        # ---- Q^T / K^T projections (feature-major) + rank-1 mean fix ----
        for xsb, wsb, cn_sb, mr_row, dst in (
            (xq_sb, wq_sb, cnq_sb, mrq, qt_sb),
            (xkv_sb, wk_sb, cnk_sb, vaug[0:1, :], kt_sb),
        ):
            for fb in range(FB):
                for g in range(2):
                    ps = prj_ps.tile([128, 1024], f32, name="prjps")
                    for half in range(2):
                        sl = slice(g * 1024 + half * 512, g * 1024 + (half + 1) * 512)
                        for kt in range(KT):
                            nc.tensor.matmul(
                                ps[:, half * 512:(half + 1) * 512],
                                wsb[:, kt, fb * 128:(fb + 1) * 128],
                                xsb[:, kt, sl],
                                start=(kt == 0), stop=False)
                        nc.tensor.matmul(
                            ps[:, half * 512:(half + 1) * 512],
                            cn_sb[:, fb, :], mr_row[:, sl],
                            start=False, stop=True)
                    nc.scalar.activation(
                        dst[:, fb, g * 1024:(g + 1) * 1024], ps, AF.Copy)

        # ---- V projection (seq-major) + rank-1 mean fix + bias ----
        for tb in range(NB16):
            sl = slice(tb * 128, (tb + 1) * 128)
            ps = v_ps.tile([128, F], f32, name="vps")
            for kt in range(KT):
                nc.tensor.matmul(ps, xkv_sb[:, kt, sl], wv_sb[:, kt, :],
                                 start=(kt == 0), stop=False)
            nc.tensor.matmul(ps, vaug[:, sl], cbv_sb, start=False, stop=True)
            nc.scalar.activation(
                v_sb[:, tb, :, 0:DK],
                ps.rearrange("p (h d) -> p h d", h=HPC), AF.Copy)
        nc.vector.memset(v_sb[:, :, :, DK:DK + 1], 1.0)

        # per-key exp bias t_k = SCALE*(bq . K_k) for all heads
        for h in range(HPC):
            fb, half = h // 2, (h % 2) * 64
            tps = pst.tile([128, NB16], f32, name="s1")
            for kb in range(NB16):
                nc.tensor.matmul(tps[:, kb:kb + 1],
                                 kt_sb[half:half + 64, fb, kb * 128:(kb + 1) * 128],
                                 bqc_sb[half:half + 64, fb:fb + 1],
                                 start=True, stop=True)
            nc.vector.tensor_scalar_mul(t_sb[:, h, :], tps, SCALE)
            nc.vector.tensor_scalar(t8_sb[:, h, :], tps, EXP_C1, EXP_B,
                                    mybir.AluOpType.mult, mybir.AluOpType.add)

    # ================= phase B: attention =================
    attn = ctx.enter_context(ExitStack())
    att_ps = attn.enter_context(tc.tile_pool(name="att", bufs=2, space="PSUM"))
    o_ps_pool = attn.enter_context(tc.tile_pool(name="ops", bufs=1, space="PSUM"))
    ptpool = attn.enter_context(tc.tile_pool(name="pt", bufs=4))
    rspool = attn.enter_context(tc.tile_pool(name="rs", bufs=4))

    for h in range(HPC):
        fb, half = h // 2, (h % 2) * 64
        opss = [o_ps_pool.tile([DK + 1, 512], f32, name=f"o{qb}")
                for qb in range(4)]
        for kb in range(NB16):
            ksl = kt_sb[half:half + 64, fb, kb * 128:(kb + 1) * 128]
            pts = []
            for t in range(2):
                sps = att_ps.tile([128, 1024], f32, name="sps")
                for g in range(2):
                    qb = 2 * t + g
                    nc.tensor.matmul(sps[:, g * 512:(g + 1) * 512], ksl,
                                     qt_sb[half:half + 64, fb, qb * 512:(qb + 1) * 512],
                                     start=True, stop=True)
                pt = ptpool.tile([128, 1024], f16, name="pt")
                if _dve_exp_tile(h, kb, t):
                    nc.vector._custom_dve(EXP8_ANT, out=pt, in0=sps,
                                          s0=t8_sb[:, h, kb:kb + 1],
                                          s1=EXP_C1, imm2=EXP_C2)
                    for _ in range(3):
                        nc.vector.tensor_mul(pt, pt, pt)
                else:
                    nc.scalar.activation(pt, sps, AF.Exp,
                                         bias=t_sb[:, h, kb:kb + 1], scale=SCALE)
                pts.append(pt)
            vsl = v_sb[:, kb, h, :]   # [128, 65]
            for qb in range(4):
                nc.tensor.matmul(opss[qb], vsl,
                                 pts[qb // 2][:, (qb % 2) * 512:(qb % 2 + 1) * 512],
                                 start=(kb == 0), stop=(kb == NB16 - 1))
        for qb in range(4):
            rs_row = rspool.tile([1, 512], f32, name="rsrow")
            nc.vector.reciprocal(rs_row, opss[qb][DK:DK + 1, :])
            rs_bc = rspool.tile([64, 512], f32, name="rsbc")
            nc.gpsimd.partition_broadcast(rs_bc, rs_row)
            nc.vector.tensor_mul(
                a_sb[half:half + 64, fb, qb * 512:(qb + 1) * 512],
                opss[qb][0:DK, :], rs_bc)

    # ================= phase C: output projection =================
    attn.close()
    op_ps = ctx.enter_context(tc.tile_pool(name="oprj", bufs=2, space="PSUM"))
    outpool = ctx.enter_context(tc.tile_pool(name="outsb", bufs=3))
    for mb in range(NB16):
        pss = [op_ps.tile([128, 384], f32, name=f"op{j}") for j in range(2)]
        for kt3 in range(FB):
            asl = a_sb[:, kt3, mb * 128:(mb + 1) * 128]
            for j in range(2):
                nc.tensor.matmul(pss[j], asl, wo_sb[:, kt3, j * 384:(j + 1) * 384],
                                 start=(kt3 == 0), stop=(kt3 == FB - 1))
        osb = outpool.tile([128, D], f16)
        for j in range(2):
            nc.vector.tensor_copy(osb[:, j * 384:(j + 1) * 384], pss[j])
        nc.sync.dma_start(out=out[mb * 128:(mb + 1) * 128, :], in_=osb)


def _build():
    nc = bacc.Bacc("TRN2", target_bir_lowering=False, debug=False, num_devices=8)
    dt = mybir.dt

    def din(name, shape, dtype):
        return nc.dram_tensor(name, list(shape), dtype, kind="ExternalInput").ap()

    io = {
        "xqT": din("xqT", (KT, 128, NSEQ), dt.float16),
        "xkvT": din("xkvT", (KT, 128, NSEQ), dt.float16),
        "wq": din("wq", (KT, 128, F), dt.float16),
        "wk": din("wk", (KT, 128, F), dt.float16),
        "wv": din("wv", (KT, 128, F), dt.float16),
        "wo": din("wo", (FB, 128, D), dt.float16),
        "cnq": din("cnq", (1, FB, 128), dt.float16),
        "cnk": din("cnk", (1, FB, 128), dt.float16),
        "cbv": din("cbv", (2, F), dt.float16),
        "bqc": din("bqc", (128, FB), dt.float16),
        "out": nc.dram_tensor("out", [NSEQ, D], dt.float16, kind="ExternalOutput").ap(),
    }

    with tile.TileContext(nc) as tc:
        with ExitStack() as ctx:
            _emit(ctx, tc, io)
    nc.compile()
    return nc


_CACHE = {}


def _get_nc():
    if "nc" not in _CACHE:
        _CACHE["nc"] = _build()
    return _CACHE["nc"]


def _prep(inputs):
    g = lambda k: np.asarray(inputs[k], dtype=np.float32)
    text, vision = g("text"), g("vision")
    ln1_w, ln1_b, ln2_w, ln2_b = g("ln1_w"), g("ln1_b"), g("ln2_w"), g("ln2_b")
    W = {nm: g("W" + nm) for nm in ("q1", "k1", "v1", "q2", "k2", "v2", "o1", "o2")}
    B = {nm: g("b" + nm) for nm in ("q1", "k1", "v1", "q2", "k2", "v2", "o1", "o2")}

    maps = [None] * 8
    for b in (0, 1):
        for path in (0, 1):
            if path == 0:
                xq, xkv = text[b], vision[b]
                lnqw, lnqb, lnkw, lnkb = ln1_w, ln1_b, ln2_w, ln2_b
                Wq, bq, Wk, Wv, bv, Wo = W["q1"], B["q1"], W["k2"], W["v2"], B["v2"], W["o1"]
            else:
                xq, xkv = vision[b], text[b]
                lnqw, lnqb, lnkw, lnkb = ln2_w, ln2_b, ln1_w, ln1_b
                Wq, bq, Wk, Wv, bv, Wo = W["q2"], B["q2"], W["k1"], W["v1"], B["v1"], W["o2"]
            xqT = np.ascontiguousarray(xq.T).astype(F16).reshape(KT, 128, NSEQ)
            xkvT = np.ascontiguousarray(xkv.T).astype(F16).reshape(KT, 128, NSEQ)
            for s in (0, 1):
                rows = slice(s * F, (s + 1) * F)
                WqT = np.ascontiguousarray(lnqw[:, None] * Wq[rows].T)
                WkT = np.ascontiguousarray(lnkw[:, None] * Wk[rows].T)
                WvT = np.ascontiguousarray(lnkw[:, None] * Wv[rows].T)
                cq = -WqT.astype(np.float32).sum(0)   # [F]
                ck = -WkT.astype(np.float32).sum(0)
                cv = -WvT.astype(np.float32).sum(0)
                bq_eff = (bq[rows] + lnqb @ Wq[rows].T).astype(np.float32)
                bv_eff = (bv[rows] + lnkb @ Wv[rows].T).astype(np.float32)
                WoT = np.ascontiguousarray(Wo[:, rows].T)  # [F, D]
                maps[b * 4 + path * 2 + s] = {
                    "xqT": xqT, "xkvT": xkvT,
                    "wq": WqT.astype(F16).reshape(KT, 128, F),
                    "wk": WkT.astype(F16).reshape(KT, 128, F),
                    "wv": WvT.astype(F16).reshape(KT, 128, F),
                    "wo": WoT.astype(F16).reshape(FB, 128, D),
                    "cnq": cq.reshape(1, FB, 128).astype(F16),
                    "cnk": ck.reshape(1, FB, 128).astype(F16),
                    "cbv": np.stack([cv, bv_eff]).astype(F16),
                    "bqc": np.ascontiguousarray(
                        bq_eff.reshape(FB, 128).T).astype(F16),
                }
    meta = (B["o1"], B["o2"])
    return maps, meta


def _unshard(results, meta):
    bo1, bo2 = meta
    text_out = np.empty((2, NSEQ, D), np.float32)
    vision_out = np.empty((2, NSEQ, D), np.float32)
    for b in (0, 1):
        text_out[b] = (results[b * 4 + 0]["out"].astype(np.float32)
                       + results[b * 4 + 1]["out"].astype(np.float32) + bo1)
        vision_out[b] = (results[b * 4 + 2]["out"].astype(np.float32)
                         + results[b * 4 + 3]["out"].astype(np.float32) + bo2)
    return (text_out, vision_out)


def run_raw(inputs, **kw):
    """Run and return the BassKernelResults (for profiling from test.py)."""
    nc = _get_nc()
    in_maps, meta = _prep(inputs)
    res = run_bass_kernel_spmd(nc, in_maps, core_ids=list(range(8)), **kw)
    return res, meta


def kernel(**inputs):
    res, meta = run_raw(inputs)
    return _unshard(res.results, meta)


# revision 14
# speedup vs baseline: 1.2188x; 1.2029x over previous
"""Trainium2 Bass kernel for a bidirectional cross-attention block.

Reference computation (per batch b):
  t = LN(text[b]);  v = LN(vision[b])
  text_out[b]   = softmax((t@Wq1.T+bq1) (v@Wk2.T+bk2)^T / 8) (v@Wv2.T+bv2) @ Wo1.T + bo1
  vision_out[b] = softmax((v@Wq2.T+bq2) (t@Wk1.T+bk1)^T / 8) (t@Wv1.T+bv1) @ Wo2.T + bo2
  (12 heads of dk=64; D=768, N=2048)

Sharding over 8 cores: (batch b in {0,1}) x (path in {text-q, vision-q}) x
(head-half in {heads 0-5, heads 6-11}).  Each core computes a [2048, 768]
partial of one output (its 6 heads pushed through the output projection);
the host sums the two head-half partials (fp16) and adds the output bias.

Device kernel (per core) highlights:
  - x passed TRANSPOSED (feature-major, fp16); LN weight folded into the
    projection weights host-side.
  - Per-token LN stats via 1-column matmuls (x tile as stationary, ones as
    moving): seq-major sums cost ~1 PE cycle each.  Stats are finished on
    tiny [128,16] tiles, then scattered to row layout by DMA.
  - LN applied as: prescale x by rstd (DVE, fp16 2x), then the mean term is
    a rank-1 PSUM update — one extra matmul row per accumulation group
    (lhsT = -colsum(W), rhs = (mu*rstd) row).  V also folds its bias via a
    ones row.  No per-tile fixup passes.
  - K-side biases dropped (softmax invariant); Q bias enters through a
    per-key correction t_k = bq . K_k folded into the exp bias.
  - exp split between ACT (native Exp) and DVE (custom EXP8 op: deg-2
    minimax parabola of e^{u/64} raised to the 8th power in one 8-stage
    pass, then 3 fp16 squarings -> e^u).  Rebalances the ACT bottleneck.
  - Row-sums of exp(S) via an appended ones-column in V; normalization
    applied after P@V (linearity).
  - Output written fp16 (halves write traffic); host sums partials in f32.
"""

import math
import os
import sys
from contextlib import ExitStack

import numpy as np

for _p in ("/opt/trn_rl_repo", os.path.expanduser("~/.axon_site/_ro/trn_rl_repo")):
    if os.path.isdir(_p) and _p not in sys.path:
        sys.path.insert(0, _p)

import ml_dtypes  # noqa: E402

import concourse.bass as bass  # noqa: E402
import concourse.bacc as bacc  # noqa: E402
import concourse.tile as tile  # noqa: E402
from concourse import mybir  # noqa: E402
from concourse import dve_ops  # noqa: E402
from concourse.dve_ops import DveOp  # noqa: E402
from concourse.dve_spec import Spec, Src0, C0, C1, C2, One, sq  # noqa: E402
from concourse.bass_utils import run_bass_kernel_spmd  # noqa: E402

F16 = np.dtype(np.float16)

NSEQ = 2048
D = 768
HEADS = 12
DK = 64
HPC = 6            # heads per core
F = HPC * DK       # 384 features per core
KT = D // 128      # 6 contraction tiles
FB = F // 128      # 3 feature blocks
NB16 = NSEQ // 128  # 16 seq blocks of 128
EPS = 1e-5
SCALE = DK ** -0.5  # 0.125

# minimax parabola e^v ~= C2*((v+B)^2+1) scaled:  w = A*v + B on |v|<=0.117
EXP_A = 1.0017179402073273
EXP_B = 1.0042971728803987
EXP_C2 = 0.4978589582950239
EXP_C1 = SCALE * EXP_A / 64.0   # multiplies the raw score


def _ref_exp8(in0, in1, c0, c1, c2):
    w = in0.astype(np.float32) * np.float32(c1) + np.asarray(c0, np.float32)
    q = (w * w + np.float32(1.0)) * np.float32(c2)
    q2 = q * q
    q4 = q2 * q2
    return q4 * q4


_w = Src0 * C1 + C0
EXP8_ANT = DveOp(
    "EXP8_ANT",
    Spec(body=sq(sq(sq((sq(_w) + One) * C2))), reference=_ref_exp8),
    subdim=False,
    uops_sha={"v3": "0772b029163394d3"},
)

if EXP8_ANT.name not in dve_ops._SUB_OPCODE_FOR_NAME:
    dve_ops.OPS.append(EXP8_ANT)
    dve_ops.CUSTOM_DVE_SPECS[EXP8_ANT.name] = EXP8_ANT.spec
    dve_ops._SUB_OPCODE_FOR_NAME[EXP8_ANT.name] = max(
        dve_ops._SUB_OPCODE_FOR_NAME.values()) + 1
    try:
        EXP8_ANT.compile("v3")
    except ValueError as e:  # sha drift: re-pin from the error message
        import re
        m = re.search(r"v3: ([0-9a-f]+)", str(e))
        if m:
            EXP8_ANT.uops_sha["v3"] = m.group(1)


def _dve_exp_tile(h, kb, t):
    """Which exp tiles run on DVE (custom op) instead of ACT."""
    return t == 1 and kb in (1, 3, 5, 7, 9, 12)


def _emit(ctx, tc, io):
    nc = tc.nc
    f32 = mybir.dt.float32
    f16 = mybir.dt.float16
    AF = mybir.ActivationFunctionType

    xqT, xkvT = io["xqT"], io["xkvT"]
    wq, wk, wv, wo = io["wq"], io["wk"], io["wv"], io["wo"]
    cnq, cnk, cbv, bqc = io["cnq"], io["cnk"], io["cbv"], io["bqc"]
    out = io["out"]

    # ---- pools that live for the whole kernel ----
    const = ctx.enter_context(tc.tile_pool(name="const", bufs=1))
    qkv_pool = ctx.enter_context(tc.tile_pool(name="qkv", bufs=1))
    wo_pool = ctx.enter_context(tc.tile_pool(name="wo", bufs=1))
    tpool = ctx.enter_context(tc.tile_pool(name="tsb", bufs=1))
    apool = ctx.enter_context(tc.tile_pool(name="afm", bufs=1))

    ones = const.tile([128, 1], f16)
    nc.vector.memset(ones, 1.0)
    eps1 = const.tile([128, 1], f32)
    nc.vector.memset(eps1, EPS)
    cnq_sb = const.tile([1, FB, 128], f16)
    nc.sync.dma_start(out=cnq_sb, in_=cnq)
    cnk_sb = const.tile([1, FB, 128], f16)
    nc.sync.dma_start(out=cnk_sb, in_=cnk)
    cbv_sb = const.tile([2, F], f16)
    nc.sync.dma_start(out=cbv_sb, in_=cbv)
    bqc_sb = const.tile([128, FB], f16)
    nc.sync.dma_start(out=bqc_sb, in_=bqc)
    # aug rows: vaug p0 = (mu*rstd)_kv, p1 = ones;  mrq p0 = (mu*rstd)_q
    vaug = const.tile([2, NSEQ], f16)
    nc.vector.memset(vaug[1:2, :], 1.0)
    mrq = const.tile([1, NSEQ], f16)
    rrow = const.tile([2, NSEQ], f16)   # p0 = rstd_q row, p1 = rstd_kv row

    qt_sb = qkv_pool.tile([128, FB, NSEQ], f16)   # Q^T feature-major
    kt_sb = qkv_pool.tile([128, FB, NSEQ], f16)   # K^T feature-major
    v_sb = qkv_pool.tile([128, NB16, HPC, DK + 1], f16)  # V seq-major + ones
    a_sb = apool.tile([128, FB, NSEQ], f16)       # attention out, feature-major

    wo_sb = wo_pool.tile([128, FB, D], f16)
    for f3 in range(FB):
        nc.sync.dma_start(out=wo_sb[:, f3, :], in_=wo[f3])

    t_sb = tpool.tile([128, HPC, NB16], f32)   # exp bias for ACT tiles
    t8_sb = tpool.tile([128, HPC, NB16], f32)  # exp bias for DVE tiles

    # ================= phase A: load x, stats, prescale, projections ========
    with ExitStack() as pre:
        xpool = pre.enter_context(tc.tile_pool(name="xt", bufs=1))
        wpool = pre.enter_context(tc.tile_pool(name="wqkv", bufs=1))
        sqpool = pre.enter_context(tc.tile_pool(name="sq", bufs=2))
        scr = pre.enter_context(tc.tile_pool(name="scratch", bufs=2))
        bcpool = pre.enter_context(tc.tile_pool(name="bc", bufs=2))
        pst = pre.enter_context(tc.tile_pool(name="pstat", bufs=1, space="PSUM"))
        prj_ps = pre.enter_context(tc.tile_pool(name="prj", bufs=2, space="PSUM"))
        v_ps = pre.enter_context(tc.tile_pool(name="vprj", bufs=2, space="PSUM"))

        xq_sb = xpool.tile([128, KT, NSEQ], f16)
        xkv_sb = xpool.tile([128, KT, NSEQ], f16)
        for kt in range(KT):
            nc.sync.dma_start(out=xkv_sb[:, kt, :], in_=xkvT[kt])
        for kt in range(KT):
            nc.sync.dma_start(out=xq_sb[:, kt, :], in_=xqT[kt])

        wq_sb = wpool.tile([128, KT, F], f16)
        wk_sb = wpool.tile([128, KT, F], f16)
        wv_sb = wpool.tile([128, KT, F], f16)
        for kt in range(KT):
            nc.sync.dma_start(out=wk_sb[:, kt, :], in_=wk[kt])
            nc.sync.dma_start(out=wv_sb[:, kt, :], in_=wv[kt])
            nc.sync.dma_start(out=wq_sb[:, kt, :], in_=wq[kt])

        # ---- per-token stats (seq-major) via 1-column matmuls; kv first ----
        rbcs = {}
        for idx, (nm, xsb) in enumerate((("kv", xkv_sb), ("q", xq_sb))):
            s1 = pst.tile([128, NB16], f32, name="s1")
            s2 = pst.tile([128, NB16], f32, name="s2")
            # token t maps to (partition t//16, column t%16) in the stat tiles
            for kt in range(KT):
                sqc = sqpool.tile([128, NSEQ], f16, name="sqc")
                nc.vector.tensor_mul(sqc, xsb[:, kt, :], xsb[:, kt, :])
                xr = xsb[:, kt, :].rearrange("p (q s) -> p s q", s=NB16)
                sr = sqc.rearrange("p (q s) -> p s q", s=NB16)
                for tb in range(NB16):
                    nc.tensor.matmul(s1[:, tb:tb + 1], xr[:, tb, :], ones,
                                     start=(kt == 0), stop=(kt == KT - 1))
                    nc.tensor.matmul(s2[:, tb:tb + 1], sr[:, tb, :], ones,
                                     start=(kt == 0), stop=(kt == KT - 1))
            mu = scr.tile([128, NB16], f32, name="mu")
            var = scr.tile([128, NB16], f32, name="var")
            nc.vector.tensor_scalar_mul(mu, s1, 1.0 / D)
            nc.vector.tensor_scalar_mul(var, s2, 1.0 / D)
            mu2 = scr.tile([128, NB16], f32, name="mu2")
            nc.vector.tensor_mul(mu2, mu, mu)
            nc.vector.tensor_sub(var, var, mu2)
            nc.scalar.activation(var, var, AF.Sqrt, bias=eps1)
            nc.vector.reciprocal(var, var)          # var now holds rstd
            nc.vector.tensor_mul(mu2, mu, var)      # mu2 now holds mu*rstd
            rt = scr.tile([128, 2 * NB16], f16, name="rt")
            nc.vector.tensor_copy(rt[:, 0:NB16], var)
            nc.vector.tensor_copy(rt[:, NB16:2 * NB16], mu2)
            # scatter to row layout: token t = p*16 + col, so p-major flatten
            # of rt[:, 0:16] is already linear token order.
            nc.sync.dma_start(out=rrow[idx:idx + 1, :], in_=rt[:, 0:NB16])
            mr_dst = (vaug[0:1, :] if nm == "kv" else mrq)
            nc.sync.dma_start(out=mr_dst, in_=rt[:, NB16:2 * NB16])
            rbc = bcpool.tile([128, NSEQ], f16, name=f"rbc_{nm}")
            nc.gpsimd.partition_broadcast(rbc, rrow[idx:idx + 1, :])
            rbcs[nm] = rbc
            # prescale this tensor right away (kv gates K/V projections)
            for kt in range(KT):
                nc.vector.tensor_mul(xsb[:, kt, :], xsb[:, kt, :], rbc)

        # ---- K^T then Q^T projections (feature-major) + rank-1 mean fix ----
        def qk_proj(xsb, wsb, cn_sb, mr_row, dst):
            for fb in range(FB):
                for g in range(2):
                    ps = prj_ps.tile([128, 1024], f32, name="prjps")
                    for half in range(2):
                        sl = slice(g * 1024 + half * 512, g * 1024 + (half + 1) * 512)
                        for kt in range(KT):
                            nc.tensor.matmul(
                                ps[:, half * 512:(half + 1) * 512],
                                wsb[:, kt, fb * 128:(fb + 1) * 128],
                                xsb[:, kt, sl],
                                start=(kt == 0), stop=False)
                        nc.tensor.matmul(
                            ps[:, half * 512:(half + 1) * 512],
                            cn_sb[:, fb, :], mr_row[:, sl],
                            start=False, stop=True)
                    nc.scalar.activation(
                        dst[:, fb, g * 1024:(g + 1) * 1024], ps, AF.Copy)

        qk_proj(xkv_sb, wk_sb, cnk_sb, vaug[0:1, :], kt_sb)

        # per-key exp bias t_k = SCALE*(bq . K_k) for all heads
        for h in range(HPC):
            fb, half = h // 2, (h % 2) * 64
            tps = pst.tile([128, NB16], f32, name="s1" if h % 2 == 0 else "s2")
            for kb in range(NB16):
                nc.tensor.matmul(tps[:, kb:kb + 1],
                                 kt_sb[half:half + 64, fb, kb * 128:(kb + 1) * 128],
                                 bqc_sb[half:half + 64, fb:fb + 1],
                                 start=True, stop=True)
            nc.vector.tensor_scalar_mul(t_sb[:, h, :], tps, SCALE)
            nc.vector.tensor_scalar(t8_sb[:, h, :], tps, EXP_C1, EXP_B,
                                    mybir.AluOpType.mult, mybir.AluOpType.add)

        # ---- V projection (seq-major) + rank-1 mean fix + bias ----
        for tb in range(NB16):
            sl = slice(tb * 128, (tb + 1) * 128)
            ps = v_ps.tile([128, F], f32, name="vps")
            for kt in range(KT):
                nc.tensor.matmul(ps, xkv_sb[:, kt, sl], wv_sb[:, kt, :],
                                 start=(kt == 0), stop=False)
            nc.tensor.matmul(ps, vaug[:, sl], cbv_sb, start=False, stop=True)
            nc.scalar.activation(
                v_sb[:, tb, :, 0:DK],
                ps.rearrange("p (h d) -> p h d", h=HPC), AF.Copy)
        nc.vector.memset(v_sb[:, :, :, DK:DK + 1], 1.0)

        qk_proj(xq_sb, wq_sb, cnq_sb, mrq, qt_sb)

    # ================= phase B: attention =================
    attn = ctx.enter_context(ExitStack())
    att_ps = attn.enter_context(tc.tile_pool(name="att", bufs=2, space="PSUM"))
    o_ps_pool = attn.enter_context(tc.tile_pool(name="ops", bufs=1, space="PSUM"))
    ptpool = attn.enter_context(tc.tile_pool(name="pt", bufs=6))
    rspool = attn.enter_context(tc.tile_pool(name="rs", bufs=4))

    PIPE = 2  # PV for kb emitted after scores for kb+PIPE (hides DVE exp chain)
    for h in range(HPC):
        fb, half = h // 2, (h % 2) * 64
        opss = [o_ps_pool.tile([DK + 1, 512], f32, name=f"o{qb}")
                for qb in range(4)]

        def emit_pv(kb, pts):
            vsl = v_sb[:, kb, h, :]   # [128, 65]
            for qb in range(4):
                nc.tensor.matmul(opss[qb], vsl,
                                 pts[qb // 2][:, (qb % 2) * 512:(qb % 2 + 1) * 512],
                                 start=(kb == 0), stop=(kb == NB16 - 1))

        pend = []
        for kb in range(NB16):
            ksl = kt_sb[half:half + 64, fb, kb * 128:(kb + 1) * 128]
            pts = []
            for t in range(2):
                sps = att_ps.tile([128, 1024], f32, name="sps")
                for g in range(2):
                    qb = 2 * t + g
                    nc.tensor.matmul(sps[:, g * 512:(g + 1) * 512], ksl,
                                     qt_sb[half:half + 64, fb, qb * 512:(qb + 1) * 512],
                                     start=True, stop=True)
                pt = ptpool.tile([128, 1024], f16, name="pt")
                if _dve_exp_tile(h, kb, t):
                    nc.vector._custom_dve(EXP8_ANT, out=pt, in0=sps,
                                          s0=t8_sb[:, h, kb:kb + 1],
                                          s1=EXP_C1, imm2=EXP_C2)
                    for _ in range(3):
                        nc.vector.tensor_mul(pt, pt, pt)
                else:
                    nc.scalar.activation(pt, sps, AF.Exp,
                                         bias=t_sb[:, h, kb:kb + 1], scale=SCALE)
                pts.append(pt)
            pend.append((kb, pts))
            if len(pend) > PIPE:
                emit_pv(*pend.pop(0))
        for item in pend:
            emit_pv(*item)
        for qb in range(4):
            rs_row = rspool.tile([1, 512], f32, name="rsrow")
            nc.vector.reciprocal(rs_row, opss[qb][DK:DK + 1, :])
            rs_bc = rspool.tile([64, 512], f32, name="rsbc")
            nc.gpsimd.partition_broadcast(rs_bc, rs_row)
            nc.vector.tensor_mul(
                a_sb[half:half + 64, fb, qb * 512:(qb + 1) * 512],
                opss[qb][0:DK, :], rs_bc)

    # ================= phase C: output projection =================
    attn.close()
    op_ps = ctx.enter_context(tc.tile_pool(name="oprj", bufs=2, space="PSUM"))
    outpool = ctx.enter_context(tc.tile_pool(name="outsb", bufs=3))
    for mb in range(NB16):
        pss = [op_ps.tile([128, 384], f32, name=f"op{j}") for j in range(2)]
        for kt3 in range(FB):
            asl = a_sb[:, kt3, mb * 128:(mb + 1) * 128]
            for j in range(2):
                nc.tensor.matmul(pss[j], asl, wo_sb[:, kt3, j * 384:(j + 1) * 384],
                                 start=(kt3 == 0), stop=(kt3 == FB - 1))
        osb = outpool.tile([128, D], f16)
        nc.vector.tensor_copy(osb[:, 0:384], pss[0])
        nc.scalar.activation(osb[:, 384:768], pss[1], AF.Copy)
        nc.sync.dma_start(out=out[mb * 128:(mb + 1) * 128, :], in_=osb)


def _build():
    nc = bacc.Bacc("TRN2", target_bir_lowering=False, debug=False, num_devices=8)
    dt = mybir.dt

    def din(name, shape, dtype):
        return nc.dram_tensor(name, list(shape), dtype, kind="ExternalInput").ap()

    io = {
        "xqT": din("xqT", (KT, 128, NSEQ), dt.float16),
        "xkvT": din("xkvT", (KT, 128, NSEQ), dt.float16),
        "wq": din("wq", (KT, 128, F), dt.float16),
        "wk": din("wk", (KT, 128, F), dt.float16),
        "wv": din("wv", (KT, 128, F), dt.float16),
        "wo": din("wo", (FB, 128, D), dt.float16),
        "cnq": din("cnq", (1, FB, 128), dt.float16),
        "cnk": din("cnk", (1, FB, 128), dt.float16),
        "cbv": din("cbv", (2, F), dt.float16),
        "bqc": din("bqc", (128, FB), dt.float16),
        "out": nc.dram_tensor("out", [NSEQ, D], dt.float16, kind="ExternalOutput").ap(),
    }

    with tile.TileContext(nc) as tc:
        with ExitStack() as ctx:
            _emit(ctx, tc, io)
    nc.compile()
    return nc


_CACHE = {}


def _get_nc():
    if "nc" not in _CACHE:
        _CACHE["nc"] = _build()
    return _CACHE["nc"]


def _prep(inputs):
    g = lambda k: np.asarray(inputs[k], dtype=np.float32)
    text, vision = g("text"), g("vision")
    ln1_w, ln1_b, ln2_w, ln2_b = g("ln1_w"), g("ln1_b"), g("ln2_w"), g("ln2_b")
    W = {nm: g("W" + nm) for nm in ("q1", "k1", "v1", "q2", "k2", "v2", "o1", "o2")}
    B = {nm: g("b" + nm) for nm in ("q1", "k1", "v1", "q2", "k2", "v2", "o1", "o2")}

    maps = [None] * 8
    for b in (0, 1):
        for path in (0, 1):
            if path == 0:
                xq, xkv = text[b], vision[b]
                lnqw, lnqb, lnkw, lnkb = ln1_w, ln1_b, ln2_w, ln2_b
                Wq, bq, Wk, Wv, bv, Wo = W["q1"], B["q1"], W["k2"], W["v2"], B["v2"], W["o1"]
            else:
                xq, xkv = vision[b], text[b]
                lnqw, lnqb, lnkw, lnkb = ln2_w, ln2_b, ln1_w, ln1_b
                Wq, bq, Wk, Wv, bv, Wo = W["q2"], B["q2"], W["k1"], W["v1"], B["v1"], W["o2"]
            xqT = np.ascontiguousarray(xq.T).astype(F16).reshape(KT, 128, NSEQ)
            xkvT = np.ascontiguousarray(xkv.T).astype(F16).reshape(KT, 128, NSEQ)
            for s in (0, 1):
                rows = slice(s * F, (s + 1) * F)
                WqT = np.ascontiguousarray(lnqw[:, None] * Wq[rows].T)
                WkT = np.ascontiguousarray(lnkw[:, None] * Wk[rows].T)
                WvT = np.ascontiguousarray(lnkw[:, None] * Wv[rows].T)
                cq = -WqT.astype(np.float32).sum(0)   # [F]
                ck = -WkT.astype(np.float32).sum(0)
                cv = -WvT.astype(np.float32).sum(0)
                bq_eff = (bq[rows] + lnqb @ Wq[rows].T).astype(np.float32)
                bv_eff = (bv[rows] + lnkb @ Wv[rows].T).astype(np.float32)
                WoT = np.ascontiguousarray(Wo[:, rows].T)  # [F, D]
                maps[b * 4 + path * 2 + s] = {
                    "xqT": xqT, "xkvT": xkvT,
                    "wq": WqT.astype(F16).reshape(KT, 128, F),
                    "wk": WkT.astype(F16).reshape(KT, 128, F),
                    "wv": WvT.astype(F16).reshape(KT, 128, F),
                    "wo": WoT.astype(F16).reshape(FB, 128, D),
                    "cnq": cq.reshape(1, FB, 128).astype(F16),
                    "cnk": ck.reshape(1, FB, 128).astype(F16),
                    "cbv": np.stack([cv, bv_eff]).astype(F16),
                    "bqc": np.ascontiguousarray(
                        bq_eff.reshape(FB, 128).T).astype(F16),
                }
    meta = (B["o1"], B["o2"])
    return maps, meta


def _unshard(results, meta):
    bo1, bo2 = meta
    text_out = np.empty((2, NSEQ, D), np.float32)
    vision_out = np.empty((2, NSEQ, D), np.float32)
    for b in (0, 1):
        text_out[b] = (results[b * 4 + 0]["out"].astype(np.float32)
                       + results[b * 4 + 1]["out"].astype(np.float32) + bo1)
        vision_out[b] = (results[b * 4 + 2]["out"].astype(np.float32)
                         + results[b * 4 + 3]["out"].astype(np.float32) + bo2)
    return (text_out, vision_out)


def run_raw(inputs, **kw):
    """Run and return the BassKernelResults (for profiling from test.py)."""
    nc = _get_nc()
    in_maps, meta = _prep(inputs)
    res = run_bass_kernel_spmd(nc, in_maps, core_ids=list(range(8)), **kw)
    return res, meta


def kernel(**inputs):
    res, meta = run_raw(inputs)
    return _unshard(res.results, meta)


# revision 22
# speedup vs baseline: 1.2405x; 1.0179x over previous
"""Trainium2 Bass kernel for a bidirectional cross-attention block.

Reference computation (per batch b):
  t = LN(text[b]);  v = LN(vision[b])
  text_out[b]   = softmax((t@Wq1.T+bq1) (v@Wk2.T+bk2)^T / 8) (v@Wv2.T+bv2) @ Wo1.T + bo1
  vision_out[b] = softmax((v@Wq2.T+bq2) (t@Wk1.T+bk1)^T / 8) (t@Wv1.T+bv1) @ Wo2.T + bo2
  (12 heads of dk=64; D=768, N=2048)

Sharding over 8 cores: (batch b in {0,1}) x (path in {text-q, vision-q}) x
(head-half in {heads 0-5, heads 6-11}).  Each core computes a [2048, 768]
partial of one output (its 6 heads pushed through the output projection);
the host sums the two head-half partials (fp16) and adds the output bias.

Device kernel (per core) highlights:
  - x passed TRANSPOSED (feature-major, fp16); LN weight folded into the
    projection weights host-side.
  - Per-token LN stats via 1-column matmuls (x tile as stationary, ones as
    moving): seq-major sums cost ~1 PE cycle each.  Stats are finished on
    tiny [128,16] tiles, then scattered to row layout by DMA.
  - LN applied as: prescale x by rstd (DVE, fp16 2x), then the mean term is
    a rank-1 PSUM update — one extra matmul row per accumulation group
    (lhsT = -colsum(W), rhs = (mu*rstd) row).  V also folds its bias via a
    ones row.  No per-tile fixup passes.
  - K-side biases dropped (softmax invariant); Q bias enters through a
    per-key correction t_k = bq . K_k folded into the exp bias.
  - exp split between ACT (native Exp) and DVE (custom EXP8 op: deg-2
    minimax parabola of e^{u/64} raised to the 8th power in one 8-stage
    pass, then 3 fp16 squarings -> e^u).  Rebalances the ACT bottleneck.
  - Row-sums of exp(S) via an appended ones-column in V; normalization
    applied after P@V (linearity).
  - Output written fp16 (halves write traffic); host sums partials in f32.
"""

import math
import os
import sys
from contextlib import ExitStack

import numpy as np

for _p in ("/opt/trn_rl_repo", os.path.expanduser("~/.axon_site/_ro/trn_rl_repo")):
    if os.path.isdir(_p) and _p not in sys.path:
        sys.path.insert(0, _p)

import ml_dtypes  # noqa: E402

import concourse.bass as bass  # noqa: E402
import concourse.bacc as bacc  # noqa: E402
import concourse.tile as tile  # noqa: E402
from concourse import mybir  # noqa: E402
from concourse import dve_ops  # noqa: E402
from concourse.dve_ops import DveOp  # noqa: E402
from concourse.dve_spec import Spec, Src0, C0, C1, C2, One, sq  # noqa: E402
from concourse.bass_utils import run_bass_kernel_spmd  # noqa: E402

F16 = np.dtype(np.float16)

NSEQ = 2048
D = 768
HEADS = 12
DK = 64
HPC = 6            # heads per core
F = HPC * DK       # 384 features per core
KT = D // 128      # 6 contraction tiles
FB = F // 128      # 3 feature blocks
NB16 = NSEQ // 128  # 16 seq blocks of 128
EPS = 1e-5
SCALE = DK ** -0.5  # 0.125

# minimax parabola e^v ~= C2*((v+B)^2+1) scaled:  w = A*v + B on |v|<=0.117
EXP_A = 1.0017179402073273
EXP_B = 1.0042971728803987
EXP_C2 = 0.4978589582950239
EXP_C1 = SCALE * EXP_A / 64.0   # multiplies the raw score


def _ref_exp8(in0, in1, c0, c1, c2):
    w = in0.astype(np.float32) * np.float32(c1) + np.asarray(c0, np.float32)
    q = (w * w + np.float32(1.0)) * np.float32(c2)
    q2 = q * q
    q4 = q2 * q2
    return q4 * q4


_w = Src0 * C1 + C0
EXP8_ANT = DveOp(
    "EXP8_ANT",
    Spec(body=sq(sq(sq((sq(_w) + One) * C2))), reference=_ref_exp8),
    subdim=False,
    uops_sha={"v3": "0772b029163394d3"},
)

if EXP8_ANT.name not in dve_ops._SUB_OPCODE_FOR_NAME:
    dve_ops.OPS.append(EXP8_ANT)
    dve_ops.CUSTOM_DVE_SPECS[EXP8_ANT.name] = EXP8_ANT.spec
    dve_ops._SUB_OPCODE_FOR_NAME[EXP8_ANT.name] = max(
        dve_ops._SUB_OPCODE_FOR_NAME.values()) + 1
    try:
        EXP8_ANT.compile("v3")
    except ValueError as e:  # sha drift: re-pin from the error message
        import re
        m = re.search(r"v3: ([0-9a-f]+)", str(e))
        if m:
            EXP8_ANT.uops_sha["v3"] = m.group(1)


def _dve_exp_tile(h, kb, t):
    """Which exp tiles run on DVE (custom op) instead of ACT."""
    return t == 1 and kb in (1, 3, 5, 7, 9, 12)


def _emit(ctx, tc, io):
    nc = tc.nc
    f32 = mybir.dt.float32
    f16 = mybir.dt.float16
    AF = mybir.ActivationFunctionType

    xqT, xkvT = io["xqT"], io["xkvT"]
    wq, wk, wv, wo = io["wq"], io["wk"], io["wv"], io["wo"]
    cnq, cnk, cbv, bqc = io["cnq"], io["cnk"], io["cbv"], io["bqc"]
    out = io["out"]

    # ---- pools that live for the whole kernel ----
    const = ctx.enter_context(tc.tile_pool(name="const", bufs=1))
    qkv_pool = ctx.enter_context(tc.tile_pool(name="qkv", bufs=1))
    wo_pool = ctx.enter_context(tc.tile_pool(name="wo", bufs=1))
    tpool = ctx.enter_context(tc.tile_pool(name="tsb", bufs=1))
    apool = ctx.enter_context(tc.tile_pool(name="afm", bufs=1))

    ones = const.tile([128, 1], f16)
    nc.vector.memset(ones, 1.0)
    eps1 = const.tile([128, 1], f32)
    nc.vector.memset(eps1, EPS)
    cnq_sb = const.tile([1, FB, 128], f16)
    nc.sync.dma_start(out=cnq_sb, in_=cnq)
    cnk_sb = const.tile([1, FB, 128], f16)
    nc.sync.dma_start(out=cnk_sb, in_=cnk)
    cbv_sb = const.tile([2, F], f16)
    nc.sync.dma_start(out=cbv_sb, in_=cbv)
    bqc_sb = const.tile([128, FB], f16)
    nc.sync.dma_start(out=bqc_sb, in_=bqc)
    # aug rows (token-indexed): vaug p0 = mu_kv, p1 = sd_kv;  mrq = mu_q
    vaug = const.tile([2, NSEQ], f16)
    mrq = const.tile([1, NSEQ], f16)
    rrow_q = const.tile([1, NSEQ], f16)   # rstd_q row
    rrow_kv = const.tile([1, NSEQ], f16)  # rstd_kv row
    rc16 = const.tile([128, NB16], f32)   # rstd_kv, seq-major by block (for V)

    qt_sb = qkv_pool.tile([128, FB, NSEQ], f16)   # Q^T feature-major
    kt_sb = qkv_pool.tile([128, FB, NSEQ], f16)   # K^T feature-major
    v_sb = qkv_pool.tile([128, NB16, HPC, DK + 1], f16)  # V seq-major + ones
    a_sb = apool.tile([128, FB, NSEQ], f16)       # attention out, feature-major

    wo_sb = wo_pool.tile([128, FB, D], f16)
    for f3 in range(FB):
        nc.sync.dma_start(out=wo_sb[:, f3, :], in_=wo[f3])

    t_sb = tpool.tile([128, HPC, NB16], f32)   # exp bias for ACT tiles
    t8_sb = tpool.tile([128, HPC, NB16], f32)  # exp bias for DVE tiles

    # ========== phase A: load x, stats, projections (post-scaled) ==========
    # LN is applied as: raw = W^T x + (-colsum)*mu (rank-1 PSUM row), then the
    # eviction multiplies by rstd — per token-column for Q/K (broadcast row),
    # per token-partition for V (tensor_scalar).  Projections therefore start
    # as soon as x/W arrive; only aug rows + evictions wait on stats.
    with ExitStack() as pre:
        xpool = pre.enter_context(tc.tile_pool(name="xt", bufs=1))
        wpool = pre.enter_context(tc.tile_pool(name="wqkv", bufs=1))
        sqpool = pre.enter_context(tc.tile_pool(name="sq", bufs=2))
        scr = pre.enter_context(tc.tile_pool(name="scratch", bufs=2))
        bcpool = pre.enter_context(tc.tile_pool(name="bc", bufs=2))
        pst = pre.enter_context(tc.tile_pool(name="pstat", bufs=1, space="PSUM"))
        prj_ps = v_ps = None  # created after kv stats (pstc needs the banks)

        xq_sb = xpool.tile([128, KT, NSEQ], f16)
        xkv_sb = xpool.tile([128, KT, NSEQ], f16)
        for kt in range(KT):
            nc.sync.dma_start(out=xkv_sb[:, kt, :], in_=xkvT[kt])

        wq_sb = wpool.tile([128, KT, F], f16)
        wk_sb = wpool.tile([128, KT, F], f16)
        wv_sb = wpool.tile([128, KT, F], f16)
        for kt in range(KT):
            nc.sync.dma_start(out=wk_sb[:, kt, :], in_=wk[kt])
            nc.sync.dma_start(out=wv_sb[:, kt, :], in_=wv[kt])
        for kt in range(KT):
            nc.sync.dma_start(out=xq_sb[:, kt, :], in_=xqT[kt])
        for kt in range(KT):
            nc.sync.dma_start(out=wq_sb[:, kt, :], in_=wq[kt])

        def finish(s1t, s2t, want_sd):
            mu = scr.tile([128, NB16], f32, name="mu")
            var = scr.tile([128, NB16], f32, name="var")
            nc.vector.tensor_scalar_mul(mu, s1t, 1.0 / D)
            nc.vector.tensor_scalar_mul(var, s2t, 1.0 / D)
            mu2 = scr.tile([128, NB16], f32, name="mu2")
            nc.vector.tensor_mul(mu2, mu, mu)
            nc.vector.tensor_sub(var, var, mu2)
            nc.scalar.activation(var, var, AF.Sqrt, bias=eps1)  # sd
            rt = scr.tile([128, 3 * NB16], f16, name="rt")
            nc.vector.tensor_copy(rt[:, NB16:2 * NB16], mu)
            if want_sd:
                nc.vector.tensor_copy(rt[:, 2 * NB16:3 * NB16], var)
            nc.vector.reciprocal(var, var)                      # rstd
            nc.vector.tensor_copy(rt[:, 0:NB16], var)
            return rt

        def stats(xsb, r_row, mu_row, sd_row, rcol):
            """Per-token LN stats.  Squares on ACT, sums via 1-col matmuls.
            p-major mapping (token = p*16 + col) feeds the row scatters;
            an extra col-major pass (token = tb*128 + p) fills rcol."""
            with ExitStack() as st:
                s1 = pst.tile([128, NB16], f32, name="s1")
                s2 = pst.tile([128, NB16], f32, name="s2")
                if rcol is not None:
                    pstc = st.enter_context(
                        tc.tile_pool(name="pstc", bufs=1, space="PSUM"))
                    c1 = pstc.tile([128, NB16], f32, name="c1")
                    c2 = pstc.tile([128, NB16], f32, name="c2")
                for kt in range(KT):
                    sqc = sqpool.tile([128, NSEQ], f16, name="sqc")
                    nc.scalar.activation(sqc, xsb[:, kt, :], AF.Square)
                    xr = xsb[:, kt, :].rearrange("p (q s) -> p s q", s=NB16)
                    sr = sqc.rearrange("p (q s) -> p s q", s=NB16)
                    for tb in range(NB16):
                        nc.tensor.matmul(s1[:, tb:tb + 1], xr[:, tb, :], ones,
                                         start=(kt == 0), stop=(kt == KT - 1))
                        nc.tensor.matmul(s2[:, tb:tb + 1], sr[:, tb, :], ones,
                                         start=(kt == 0), stop=(kt == KT - 1))
                        if rcol is not None:
                            sl = slice(tb * 128, (tb + 1) * 128)
                            nc.tensor.matmul(c1[:, tb:tb + 1],
                                             xsb[:, kt, sl], ones,
                                             start=(kt == 0), stop=(kt == KT - 1))
                            nc.tensor.matmul(c2[:, tb:tb + 1], sqc[:, sl], ones,
                                             start=(kt == 0), stop=(kt == KT - 1))
                rt = finish(s1, s2, sd_row is not None)
                # p-major flatten of a 16-col block is linear token order
                nc.gpsimd.dma_start(out=r_row, in_=rt[:, 0:NB16])
                nc.gpsimd.dma_start(out=mu_row, in_=rt[:, NB16:2 * NB16])
                if sd_row is not None:
                    nc.gpsimd.dma_start(out=sd_row, in_=rt[:, 2 * NB16:3 * NB16])
                if rcol is not None:
                    mu_c = scr.tile([128, NB16], f32, name="mu")
                    var_c = scr.tile([128, NB16], f32, name="var")
                    nc.vector.tensor_scalar_mul(mu_c, c1, 1.0 / D)
                    nc.vector.tensor_scalar_mul(var_c, c2, 1.0 / D)
                    mu2_c = scr.tile([128, NB16], f32, name="mu2")
                    nc.vector.tensor_mul(mu2_c, mu_c, mu_c)
                    nc.vector.tensor_sub(var_c, var_c, mu2_c)
                    nc.scalar.activation(var_c, var_c, AF.Sqrt, bias=eps1)
                    nc.vector.reciprocal(rcol, var_c)

        def qk_proj(xsb, wsb, cn_sb, mu_row, r_bc, dst):
            for fb in range(FB):
                for g in range(2):
                    ps = prj_ps.tile([128, 1024], f32, name="prjps")
                    gsl = slice(g * 1024, (g + 1) * 1024)
                    for half in range(2):
                        sl = slice(g * 1024 + half * 512, g * 1024 + (half + 1) * 512)
                        for kt in range(KT):
                            nc.tensor.matmul(
                                ps[:, half * 512:(half + 1) * 512],
                                wsb[:, kt, fb * 128:(fb + 1) * 128],
                                xsb[:, kt, sl],
                                start=(kt == 0), stop=False)
                        nc.tensor.matmul(
                            ps[:, half * 512:(half + 1) * 512],
                            cn_sb[:, fb, :], mu_row[:, sl],
                            start=False, stop=True)
                    nc.vector.tensor_mul(dst[:, fb, gsl], ps, r_bc[:, gsl])

        # ---- kv side: stats -> K projection -> t_sb -> V projection ----
        stats(xkv_sb, rrow_kv, vaug[0:1, :], vaug[1:2, :], rc16)
        prj_ps = pre.enter_context(tc.tile_pool(name="prj", bufs=2, space="PSUM"))
        v_ps = pre.enter_context(tc.tile_pool(name="vprj", bufs=2, space="PSUM"))
        rbc_kv = bcpool.tile([128, NSEQ], f16, name="rbc_kv")
        nc.gpsimd.partition_broadcast(rbc_kv, rrow_kv)
        qk_proj(xkv_sb, wk_sb, cnk_sb, vaug[0:1, :], rbc_kv, kt_sb)

        # per-key exp bias t_k = SCALE*(bq . K_k) for all heads
        for h in range(HPC):
            fb, half = h // 2, (h % 2) * 64
            tps = pst.tile([128, NB16], f32, name="s1" if h % 2 == 0 else "s2")
            for kb in range(NB16):
                nc.tensor.matmul(tps[:, kb:kb + 1],
                                 kt_sb[half:half + 64, fb, kb * 128:(kb + 1) * 128],
                                 bqc_sb[half:half + 64, fb:fb + 1],
                                 start=True, stop=True)
            nc.vector.tensor_scalar_mul(t_sb[:, h, :], tps, SCALE)
            nc.vector.tensor_scalar(t8_sb[:, h, :], tps, EXP_C1, EXP_B,
                                    mybir.AluOpType.mult, mybir.AluOpType.add)

        # ---- q side: stats -> Q projection ----
        stats(xq_sb, rrow_q, mrq, None, None)
        rbc_q = bcpool.tile([128, NSEQ], f16, name="rbc_q")
        nc.gpsimd.partition_broadcast(rbc_q, rrow_q)
        qk_proj(xq_sb, wq_sb, cnq_sb, mrq, rbc_q, qt_sb)

        # ---- V projection (seq-major): aug rows [mu; sd] x [-colsum; bv],
        # eviction scales by rstd per token partition (sd*rstd = 1). ----
        for tb in range(NB16):
            sl = slice(tb * 128, (tb + 1) * 128)
            ps = v_ps.tile([128, F], f32, name="vps")
            for kt in range(KT):
                nc.tensor.matmul(ps, xkv_sb[:, kt, sl], wv_sb[:, kt, :],
                                 start=(kt == 0), stop=False)
            nc.tensor.matmul(ps, vaug[:, sl], cbv_sb, start=False, stop=True)
            nc.vector.tensor_scalar_mul(
                v_sb[:, tb, :, 0:DK],
                ps.rearrange("p (h d) -> p h d", h=HPC), rc16[:, tb:tb + 1])
        nc.vector.memset(v_sb[:, :, :, DK:DK + 1], 1.0)

    # ================= phase B: attention =================
    attn = ctx.enter_context(ExitStack())
    att_ps = attn.enter_context(tc.tile_pool(name="att", bufs=2, space="PSUM"))
    o_ps_pool = attn.enter_context(tc.tile_pool(name="ops", bufs=1, space="PSUM"))
    ptpool = attn.enter_context(tc.tile_pool(name="pt", bufs=6))
    rspool = attn.enter_context(tc.tile_pool(name="rs", bufs=4))

    PIPE = 2  # PV for kb emitted after scores for kb+PIPE (hides DVE exp chain)
    for h in range(HPC):
        fb, half = h // 2, (h % 2) * 64
        opss = [o_ps_pool.tile([DK + 1, 512], f32, name=f"o{qb}")
                for qb in range(4)]

        def emit_pv(kb, pts):
            vsl = v_sb[:, kb, h, :]   # [128, 65]
            for qb in range(4):
                nc.tensor.matmul(opss[qb], vsl,
                                 pts[qb // 2][:, (qb % 2) * 512:(qb % 2 + 1) * 512],
                                 start=(kb == 0), stop=(kb == NB16 - 1))

        pend = []
        for kb in range(NB16):
            ksl = kt_sb[half:half + 64, fb, kb * 128:(kb + 1) * 128]
            pts = []
            for t in range(2):
                sps = att_ps.tile([128, 1024], f32, name="sps")
                for g in range(2):
                    qb = 2 * t + g
                    nc.tensor.matmul(sps[:, g * 512:(g + 1) * 512], ksl,
                                     qt_sb[half:half + 64, fb, qb * 512:(qb + 1) * 512],
                                     start=True, stop=True)
                pt = ptpool.tile([128, 1024], f16, name="pt")
                if _dve_exp_tile(h, kb, t):
                    nc.vector._custom_dve(EXP8_ANT, out=pt, in0=sps,
                                          s0=t8_sb[:, h, kb:kb + 1],
                                          s1=EXP_C1, imm2=EXP_C2)
                    for _ in range(3):
                        nc.vector.tensor_mul(pt, pt, pt)
                else:
                    nc.scalar.activation(pt, sps, AF.Exp,
                                         bias=t_sb[:, h, kb:kb + 1], scale=SCALE)
                pts.append(pt)
            pend.append((kb, pts))
            if len(pend) > PIPE:
                emit_pv(*pend.pop(0))
        for item in pend:
            emit_pv(*item)
        for qb in range(4):
            rs_row = rspool.tile([1, 512], f32, name="rsrow")
            nc.vector.reciprocal(rs_row, opss[qb][DK:DK + 1, :])
            rs_bc = rspool.tile([64, 512], f32, name="rsbc")
            nc.gpsimd.partition_broadcast(rs_bc, rs_row)
            nc.vector.tensor_mul(
                a_sb[half:half + 64, fb, qb * 512:(qb + 1) * 512],
                opss[qb][0:DK, :], rs_bc)

    # ================= phase C: output projection =================
    attn.close()
    op_ps = ctx.enter_context(tc.tile_pool(name="oprj", bufs=2, space="PSUM"))
    outpool = ctx.enter_context(tc.tile_pool(name="outsb", bufs=3))
    for mb in range(NB16):
        pss = [op_ps.tile([128, 384], f32, name=f"op{j}") for j in range(2)]
        for kt3 in range(FB):
            asl = a_sb[:, kt3, mb * 128:(mb + 1) * 128]
            for j in range(2):
                nc.tensor.matmul(pss[j], asl, wo_sb[:, kt3, j * 384:(j + 1) * 384],
                                 start=(kt3 == 0), stop=(kt3 == FB - 1))
        osb = outpool.tile([128, D], f16)
        nc.vector.tensor_copy(osb[:, 0:384], pss[0])
        nc.scalar.activation(osb[:, 384:768], pss[1], AF.Copy)
        nc.sync.dma_start(out=out[mb * 128:(mb + 1) * 128, :], in_=osb)


def _build():
    nc = bacc.Bacc("TRN2", target_bir_lowering=False, debug=False, num_devices=8)
    dt = mybir.dt

    def din(name, shape, dtype):
        return nc.dram_tensor(name, list(shape), dtype, kind="ExternalInput").ap()

    io = {
        "xqT": din("xqT", (KT, 128, NSEQ), dt.float16),
        "xkvT": din("xkvT", (KT, 128, NSEQ), dt.float16),
        "wq": din("wq", (KT, 128, F), dt.float16),
        "wk": din("wk", (KT, 128, F), dt.float16),
        "wv": din("wv", (KT, 128, F), dt.float16),
        "wo": din("wo", (FB, 128, D), dt.float16),
        "cnq": din("cnq", (1, FB, 128), dt.float16),
        "cnk": din("cnk", (1, FB, 128), dt.float16),
        "cbv": din("cbv", (2, F), dt.float16),
        "bqc": din("bqc", (128, FB), dt.float16),
        "out": nc.dram_tensor("out", [NSEQ, D], dt.float16, kind="ExternalOutput").ap(),
    }

    with tile.TileContext(nc) as tc:
        with ExitStack() as ctx:
            _emit(ctx, tc, io)
    nc.compile()
    return nc


_CACHE = {}


def _get_nc():
    if "nc" not in _CACHE:
        _CACHE["nc"] = _build()
    return _CACHE["nc"]


def _prep(inputs):
    g = lambda k: np.asarray(inputs[k], dtype=np.float32)
    text, vision = g("text"), g("vision")
    ln1_w, ln1_b, ln2_w, ln2_b = g("ln1_w"), g("ln1_b"), g("ln2_w"), g("ln2_b")
    W = {nm: g("W" + nm) for nm in ("q1", "k1", "v1", "q2", "k2", "v2", "o1", "o2")}
    B = {nm: g("b" + nm) for nm in ("q1", "k1", "v1", "q2", "k2", "v2", "o1", "o2")}

    maps = [None] * 8
    for b in (0, 1):
        for path in (0, 1):
            if path == 0:
                xq, xkv = text[b], vision[b]
                lnqw, lnqb, lnkw, lnkb = ln1_w, ln1_b, ln2_w, ln2_b
                Wq, bq, Wk, Wv, bv, Wo = W["q1"], B["q1"], W["k2"], W["v2"], B["v2"], W["o1"]
            else:
                xq, xkv = vision[b], text[b]
                lnqw, lnqb, lnkw, lnkb = ln2_w, ln2_b, ln1_w, ln1_b
                Wq, bq, Wk, Wv, bv, Wo = W["q2"], B["q2"], W["k1"], W["v1"], B["v1"], W["o2"]
            xqT = np.ascontiguousarray(xq.T).astype(F16).reshape(KT, 128, NSEQ)
            xkvT = np.ascontiguousarray(xkv.T).astype(F16).reshape(KT, 128, NSEQ)
            for s in (0, 1):
                rows = slice(s * F, (s + 1) * F)
                WqT = np.ascontiguousarray(lnqw[:, None] * Wq[rows].T)
                WkT = np.ascontiguousarray(lnkw[:, None] * Wk[rows].T)
                WvT = np.ascontiguousarray(lnkw[:, None] * Wv[rows].T)
                cq = -WqT.astype(np.float32).sum(0)   # [F]
                ck = -WkT.astype(np.float32).sum(0)
                cv = -WvT.astype(np.float32).sum(0)
                bq_eff = (bq[rows] + lnqb @ Wq[rows].T).astype(np.float32)
                bv_eff = (bv[rows] + lnkb @ Wv[rows].T).astype(np.float32)
                WoT = np.ascontiguousarray(Wo[:, rows].T)  # [F, D]
                maps[b * 4 + path * 2 + s] = {
                    "xqT": xqT, "xkvT": xkvT,
                    "wq": WqT.astype(F16).reshape(KT, 128, F),
                    "wk": WkT.astype(F16).reshape(KT, 128, F),
                    "wv": WvT.astype(F16).reshape(KT, 128, F),
                    "wo": WoT.astype(F16).reshape(FB, 128, D),
                    "cnq": cq.reshape(1, FB, 128).astype(F16),
                    "cnk": ck.reshape(1, FB, 128).astype(F16),
                    "cbv": np.stack([cv, bv_eff]).astype(F16),
                    "bqc": np.ascontiguousarray(
                        bq_eff.reshape(FB, 128).T).astype(F16),
                }
    meta = (B["o1"], B["o2"])
    return maps, meta


def _unshard(results, meta):
    bo1, bo2 = meta
    text_out = np.empty((2, NSEQ, D), np.float32)
    vision_out = np.empty((2, NSEQ, D), np.float32)
    for b in (0, 1):
        text_out[b] = (results[b * 4 + 0]["out"].astype(np.float32)
                       + results[b * 4 + 1]["out"].astype(np.float32) + bo1)
        vision_out[b] = (results[b * 4 + 2]["out"].astype(np.float32)
                         + results[b * 4 + 3]["out"].astype(np.float32) + bo2)
    return (text_out, vision_out)


def run_raw(inputs, **kw):
    """Run and return the BassKernelResults (for profiling from test.py)."""
    nc = _get_nc()
    in_maps, meta = _prep(inputs)
    res = run_bass_kernel_spmd(nc, in_maps, core_ids=list(range(8)), **kw)
    return res, meta


def kernel(**inputs):
    res, meta = run_raw(inputs)
    return _unshard(res.results, meta)


# revision 29
# speedup vs baseline: 1.2560x; 1.0125x over previous
"""Trainium2 Bass kernel for a bidirectional cross-attention block.

Reference computation (per batch b):
  t = LN(text[b]);  v = LN(vision[b])
  text_out[b]   = softmax((t@Wq1.T+bq1) (v@Wk2.T+bk2)^T / 8) (v@Wv2.T+bv2) @ Wo1.T + bo1
  vision_out[b] = softmax((v@Wq2.T+bq2) (t@Wk1.T+bk1)^T / 8) (t@Wv1.T+bv1) @ Wo2.T + bo2
  (12 heads of dk=64; D=768, N=2048)

Sharding over 8 cores: (batch b in {0,1}) x (path in {text-q, vision-q}) x
(head-half in {heads 0-5, heads 6-11}).  Each core computes a [2048, 768]
partial of one output (its 6 heads pushed through the output projection);
the host sums the two head-half partials (fp16) and adds the output bias.

Device kernel (per core) highlights:
  - x passed TRANSPOSED (feature-major, fp16); LN weight folded into the
    projection weights host-side.
  - Per-token LN stats via 1-column matmuls (x tile as stationary, ones as
    moving): seq-major sums cost ~1 PE cycle each.  Stats are finished on
    tiny [128,16] tiles, then scattered to row layout by DMA.
  - LN applied as: prescale x by rstd (DVE, fp16 2x), then the mean term is
    a rank-1 PSUM update — one extra matmul row per accumulation group
    (lhsT = -colsum(W), rhs = (mu*rstd) row).  V also folds its bias via a
    ones row.  No per-tile fixup passes.
  - K-side biases dropped (softmax invariant); Q bias enters through a
    per-key correction t_k = bq . K_k folded into the exp bias.
  - exp split between ACT (native Exp) and DVE (custom EXP8 op: deg-2
    minimax parabola of e^{u/64} raised to the 8th power in one 8-stage
    pass, then 3 fp16 squarings -> e^u).  Rebalances the ACT bottleneck.
  - Row-sums of exp(S) via an appended ones-column in V; normalization
    applied after P@V (linearity).
  - Output written fp16 (halves write traffic); host sums partials in f32.
"""

import math
import os
import sys
from contextlib import ExitStack

import numpy as np

for _p in ("/opt/trn_rl_repo", os.path.expanduser("~/.axon_site/_ro/trn_rl_repo")):
    if os.path.isdir(_p) and _p not in sys.path:
        sys.path.insert(0, _p)

import ml_dtypes  # noqa: E402

import concourse.bass as bass  # noqa: E402
import concourse.bacc as bacc  # noqa: E402
import concourse.tile as tile  # noqa: E402
from concourse import mybir  # noqa: E402
from concourse import dve_ops  # noqa: E402
from concourse.dve_ops import DveOp  # noqa: E402
from concourse.dve_spec import Spec, Src0, C0, C1, C2, One, sq  # noqa: E402
from concourse.bass_utils import run_bass_kernel_spmd  # noqa: E402

F16 = np.dtype(np.float16)

NSEQ = 2048
D = 768
HEADS = 12
DK = 64
HPC = 6            # heads per core
F = HPC * DK       # 384 features per core
KT = D // 128      # 6 contraction tiles
FB = F // 128      # 3 feature blocks
NB16 = NSEQ // 128  # 16 seq blocks of 128
EPS = 1e-5
SCALE = DK ** -0.5  # 0.125

# minimax parabola e^v ~= C2*((v+B)^2+1) scaled:  w = A*v + B on |v|<=0.117
EXP_A = 1.0017179402073273
EXP_B = 1.0042971728803987
EXP_C2 = 0.4978589582950239
EXP_C1 = SCALE * EXP_A / 64.0   # multiplies the raw score


def _ref_exp8(in0, in1, c0, c1, c2):
    w = in0.astype(np.float32) * np.float32(c1) + np.asarray(c0, np.float32)
    q = (w * w + np.float32(1.0)) * np.float32(c2)
    q2 = q * q
    q4 = q2 * q2
    return q4 * q4


_w = Src0 * C1 + C0
EXP8_ANT = DveOp(
    "EXP8_ANT",
    Spec(body=sq(sq(sq((sq(_w) + One) * C2))), reference=_ref_exp8),
    subdim=False,
    uops_sha={"v3": "0772b029163394d3"},
)

if EXP8_ANT.name not in dve_ops._SUB_OPCODE_FOR_NAME:
    dve_ops.OPS.append(EXP8_ANT)
    dve_ops.CUSTOM_DVE_SPECS[EXP8_ANT.name] = EXP8_ANT.spec
    dve_ops._SUB_OPCODE_FOR_NAME[EXP8_ANT.name] = max(
        dve_ops._SUB_OPCODE_FOR_NAME.values()) + 1
    try:
        EXP8_ANT.compile("v3")
    except ValueError as e:  # sha drift: re-pin from the error message
        import re
        m = re.search(r"v3: ([0-9a-f]+)", str(e))
        if m:
            EXP8_ANT.uops_sha["v3"] = m.group(1)


def _dve_exp_tile(h, kb, t):
    """Which exp tiles run on DVE (custom op) instead of ACT."""
    return t == 1 and kb in (1, 3, 5, 7, 9, 11, 13)


def _emit(ctx, tc, io):
    nc = tc.nc
    f32 = mybir.dt.float32
    f16 = mybir.dt.float16
    AF = mybir.ActivationFunctionType

    xqT, xkvT = io["xqT"], io["xkvT"]
    wq, wk, wv, wo = io["wq"], io["wk"], io["wv"], io["wo"]
    cnq, cnk, cbv, bqc = io["cnq"], io["cnk"], io["cbv"], io["bqc"]
    out = io["out"]

    # ---- pools that live for the whole kernel ----
    const = ctx.enter_context(tc.tile_pool(name="const", bufs=1))
    qkv_pool = ctx.enter_context(tc.tile_pool(name="qkv", bufs=1))
    wo_pool = ctx.enter_context(tc.tile_pool(name="wo", bufs=1))
    tpool = ctx.enter_context(tc.tile_pool(name="tsb", bufs=1))
    apool = ctx.enter_context(tc.tile_pool(name="afm", bufs=1))

    ones = const.tile([128, 1], f16)
    nc.vector.memset(ones, 1.0)
    eps1 = const.tile([128, 1], f32)
    nc.vector.memset(eps1, EPS)
    cnq_sb = const.tile([1, FB, 128], f16)
    nc.sync.dma_start(out=cnq_sb, in_=cnq)
    cnk_sb = const.tile([1, FB, 128], f16)
    nc.sync.dma_start(out=cnk_sb, in_=cnk)
    cbv_sb = const.tile([2, F], f16)
    nc.sync.dma_start(out=cbv_sb, in_=cbv)
    bqc_sb = const.tile([128, FB], f16)
    nc.sync.dma_start(out=bqc_sb, in_=bqc)
    # aug rows (token-indexed): vaug p0 = mu_kv, p1 = sd_kv;  mrq = mu_q
    vaug = const.tile([2, NSEQ], f16)
    mrq = const.tile([1, NSEQ], f16)
    rrow_q = const.tile([1, NSEQ], f16)   # rstd_q row
    rrow_kv = const.tile([1, NSEQ], f16)  # rstd_kv row
    rc16 = const.tile([128, NB16], f32)   # rstd_kv, seq-major by block (for V)

    qt_sb = qkv_pool.tile([128, FB, NSEQ], f16)   # Q^T feature-major
    kt_sb = qkv_pool.tile([128, FB, NSEQ], f16)   # K^T feature-major
    v_sb = qkv_pool.tile([128, NB16, HPC, DK + 1], f16)  # V seq-major + ones
    a_sb = apool.tile([128, FB, NSEQ], f16)       # attention out, feature-major

    wo_sb = wo_pool.tile([128, FB, D], f16)
    for f3 in range(FB):
        nc.sync.dma_start(out=wo_sb[:, f3, :], in_=wo[f3])

    t_sb = tpool.tile([128, HPC, NB16], f32)   # exp bias for ACT tiles
    t8_sb = tpool.tile([128, HPC, NB16], f32)  # exp bias for DVE tiles

    # ========== phase A: load x, stats, projections (post-scaled) ==========
    # LN is applied as: raw = W^T x + (-colsum)*mu (rank-1 PSUM row), then the
    # eviction multiplies by rstd — per token-column for Q/K (broadcast row),
    # per token-partition for V (tensor_scalar).  Projections therefore start
    # as soon as x/W arrive; only aug rows + evictions wait on stats.
    with ExitStack() as pre:
        xpool = pre.enter_context(tc.tile_pool(name="xt", bufs=1))
        wpool = pre.enter_context(tc.tile_pool(name="wqkv", bufs=1))
        sqpool = pre.enter_context(tc.tile_pool(name="sq", bufs=6))
        scr = pre.enter_context(tc.tile_pool(name="scratch", bufs=2))
        bcpool = pre.enter_context(tc.tile_pool(name="bc", bufs=2))
        pst = pre.enter_context(tc.tile_pool(name="pstat", bufs=1, space="PSUM"))
        prj_ps = v_ps = None  # created after kv stats (pstc needs the banks)

        xq_sb = xpool.tile([128, KT, NSEQ], f16)
        xkv_sb = xpool.tile([128, KT, NSEQ], f16)
        for kt in range(KT):
            nc.sync.dma_start(out=xkv_sb[:, kt, :], in_=xkvT[kt])

        wq_sb = wpool.tile([128, KT, F], f16)
        wk_sb = wpool.tile([128, KT, F], f16)
        wv_sb = wpool.tile([128, KT, F], f16)
        for kt in range(KT):
            nc.sync.dma_start(out=wk_sb[:, kt, :], in_=wk[kt])
        for kt in range(KT):
            nc.sync.dma_start(out=xq_sb[:, kt, :], in_=xqT[kt])
        for kt in range(KT):
            nc.sync.dma_start(out=wv_sb[:, kt, :], in_=wv[kt])
            nc.sync.dma_start(out=wq_sb[:, kt, :], in_=wq[kt])

        def finish(s1t, s2t, want_sd):
            mu = scr.tile([128, NB16], f32, name="mu")
            var = scr.tile([128, NB16], f32, name="var")
            nc.vector.tensor_scalar_mul(mu, s1t, 1.0 / D)
            nc.vector.tensor_scalar_mul(var, s2t, 1.0 / D)
            mu2 = scr.tile([128, NB16], f32, name="mu2")
            nc.vector.tensor_mul(mu2, mu, mu)
            nc.vector.tensor_sub(var, var, mu2)
            nc.scalar.activation(var, var, AF.Sqrt, bias=eps1)  # sd
            rt = scr.tile([128, 3 * NB16], f16, name="rt")
            nc.vector.tensor_copy(rt[:, NB16:2 * NB16], mu)
            if want_sd:
                nc.vector.tensor_copy(rt[:, 2 * NB16:3 * NB16], var)
            nc.vector.reciprocal(var, var)                      # rstd
            nc.vector.tensor_copy(rt[:, 0:NB16], var)
            return rt

        def stats(xsb, r_row, mu_row, sd_row, rcol):
            """Per-token LN stats.  Squares on ACT, sums via 1-col matmuls.
            p-major mapping (token = p*16 + col) feeds the row scatters;
            an extra col-major pass (token = tb*128 + p) fills rcol."""
            with ExitStack() as st:
                s1 = pst.tile([128, NB16], f32, name="s1")
                s2 = pst.tile([128, NB16], f32, name="s2")
                if rcol is not None:
                    pstc = st.enter_context(
                        tc.tile_pool(name="pstc", bufs=1, space="PSUM"))
                    c1 = pstc.tile([128, NB16], f32, name="c1")
                    c2 = pstc.tile([128, NB16], f32, name="c2")
                # PSUM zero-regions are whole banks: a group's start marks the
                # entire bank, so each column's 6-step accumulation must fully
                # complete before the next column's start -> kt INNER loop.
                sqcs = []
                for kt in range(KT):
                    sqc = sqpool.tile([128, NSEQ], f16, name="sqc")
                    if kt % 2 == 0:
                        nc.scalar.activation(sqc, xsb[:, kt, :], AF.Square)
                    else:
                        nc.vector.tensor_mul(sqc, xsb[:, kt, :], xsb[:, kt, :])
                    sqcs.append(sqc)
                for tb in range(NB16):
                    sl = slice(tb * 128, (tb + 1) * 128)
                    for kt in range(KT):
                        xr = xsb[:, kt, :].rearrange("p (q s) -> p s q", s=NB16)
                        sr = sqcs[kt].rearrange("p (q s) -> p s q", s=NB16)
                        nc.tensor.matmul(s1[:, tb:tb + 1], xr[:, tb, :], ones,
                                         start=(kt == 0), stop=(kt == KT - 1))
                        nc.tensor.matmul(s2[:, tb:tb + 1], sr[:, tb, :], ones,
                                         start=(kt == 0), stop=(kt == KT - 1))
                        if rcol is not None:
                            nc.tensor.matmul(c1[:, tb:tb + 1],
                                             xsb[:, kt, sl], ones,
                                             start=(kt == 0), stop=(kt == KT - 1))
                            nc.tensor.matmul(c2[:, tb:tb + 1], sqcs[kt][:, sl],
                                             ones,
                                             start=(kt == 0), stop=(kt == KT - 1))
                rt = finish(s1, s2, sd_row is not None)
                # p-major flatten of a 16-col block is linear token order
                nc.gpsimd.dma_start(out=r_row, in_=rt[:, 0:NB16])
                nc.gpsimd.dma_start(out=mu_row, in_=rt[:, NB16:2 * NB16])
                if sd_row is not None:
                    nc.gpsimd.dma_start(out=sd_row, in_=rt[:, 2 * NB16:3 * NB16])
                if rcol is not None:
                    mu_c = scr.tile([128, NB16], f32, name="mu")
                    var_c = scr.tile([128, NB16], f32, name="var")
                    nc.vector.tensor_scalar_mul(mu_c, c1, 1.0 / D)
                    nc.vector.tensor_scalar_mul(var_c, c2, 1.0 / D)
                    mu2_c = scr.tile([128, NB16], f32, name="mu2")
                    nc.vector.tensor_mul(mu2_c, mu_c, mu_c)
                    nc.vector.tensor_sub(var_c, var_c, mu2_c)
                    nc.scalar.activation(var_c, var_c, AF.Sqrt, bias=eps1)
                    nc.vector.reciprocal(rcol, var_c)

        def qk_proj(xsb, wsb, cn_sb, mu_row, r_bc, dst):
            for fb in range(FB):
                for g in range(2):
                    ps = prj_ps.tile([128, 1024], f32, name="prjps")
                    gsl = slice(g * 1024, (g + 1) * 1024)
                    for half in range(2):
                        sl = slice(g * 1024 + half * 512, g * 1024 + (half + 1) * 512)
                        for kt in range(KT):
                            nc.tensor.matmul(
                                ps[:, half * 512:(half + 1) * 512],
                                wsb[:, kt, fb * 128:(fb + 1) * 128],
                                xsb[:, kt, sl],
                                start=(kt == 0), stop=False)
                        nc.tensor.matmul(
                            ps[:, half * 512:(half + 1) * 512],
                            cn_sb[:, fb, :], mu_row[:, sl],
                            start=False, stop=True)
                    nc.vector.tensor_mul(dst[:, fb, gsl], ps, r_bc[:, gsl])

        # ---- kv side: stats -> K projection -> t_sb -> V projection ----
        stats(xkv_sb, rrow_kv, vaug[0:1, :], vaug[1:2, :], rc16)
        prj_ps = pre.enter_context(tc.tile_pool(name="prj", bufs=2, space="PSUM"))
        v_ps = pre.enter_context(tc.tile_pool(name="vprj", bufs=2, space="PSUM"))
        rbc_kv = bcpool.tile([128, NSEQ], f16, name="rbc_kv")
        nc.gpsimd.partition_broadcast(rbc_kv, rrow_kv)
        qk_proj(xkv_sb, wk_sb, cnk_sb, vaug[0:1, :], rbc_kv, kt_sb)

        # per-key exp bias t_k = SCALE*(bq . K_k) for all heads
        for h in range(HPC):
            fb, half = h // 2, (h % 2) * 64
            tps = pst.tile([128, NB16], f32, name="s1" if h % 2 == 0 else "s2")
            for kb in range(NB16):
                nc.tensor.matmul(tps[:, kb:kb + 1],
                                 kt_sb[half:half + 64, fb, kb * 128:(kb + 1) * 128],
                                 bqc_sb[half:half + 64, fb:fb + 1],
                                 start=True, stop=True)
            nc.vector.tensor_scalar_mul(t_sb[:, h, :], tps, SCALE)
            nc.vector.tensor_scalar(t8_sb[:, h, :], tps, EXP_C1, EXP_B,
                                    mybir.AluOpType.mult, mybir.AluOpType.add)

        # ---- q side: stats -> Q projection ----
        stats(xq_sb, rrow_q, mrq, None, None)
        rbc_q = bcpool.tile([128, NSEQ], f16, name="rbc_q")
        nc.gpsimd.partition_broadcast(rbc_q, rrow_q)
        qk_proj(xq_sb, wq_sb, cnq_sb, mrq, rbc_q, qt_sb)

        # ---- V projection (seq-major): aug rows [mu; sd] x [-colsum; bv],
        # eviction scales by rstd per token partition (sd*rstd = 1). ----
        for tb in range(NB16):
            sl = slice(tb * 128, (tb + 1) * 128)
            ps = v_ps.tile([128, F], f32, name="vps")
            for kt in range(KT):
                nc.tensor.matmul(ps, xkv_sb[:, kt, sl], wv_sb[:, kt, :],
                                 start=(kt == 0), stop=False)
            nc.tensor.matmul(ps, vaug[:, sl], cbv_sb, start=False, stop=True)
            nc.vector.tensor_scalar_mul(
                v_sb[:, tb, :, 0:DK],
                ps.rearrange("p (h d) -> p h d", h=HPC), rc16[:, tb:tb + 1])
        nc.vector.memset(v_sb[:, :, :, DK:DK + 1], 1.0)

    if "dq" in io:  # debug dumps
        nc.sync.dma_start(out=io["dq"], in_=qt_sb)
        nc.sync.dma_start(out=io["dk"], in_=kt_sb)
        nc.sync.dma_start(out=io["dv"], in_=v_sb)

    # ================= phase B: attention =================
    attn = ctx.enter_context(ExitStack())
    att_ps = attn.enter_context(tc.tile_pool(name="att", bufs=2, space="PSUM"))
    o_ps_pool = attn.enter_context(tc.tile_pool(name="ops", bufs=1, space="PSUM"))
    ptpool = attn.enter_context(tc.tile_pool(name="pt", bufs=8))
    rspool = attn.enter_context(tc.tile_pool(name="rs", bufs=4))

    PIPE = 3  # PV for kb emitted after scores for kb+PIPE (hides DVE exp chain)
    for h in range(HPC):
        fb, half = h // 2, (h % 2) * 64
        opss = [o_ps_pool.tile([DK + 1, 512], f32, name=f"o{qb}")
                for qb in range(4)]

        def emit_pv(kb, pts):
            vsl = v_sb[:, kb, h, :]   # [128, 65]
            for qb in range(4):
                nc.tensor.matmul(opss[qb], vsl,
                                 pts[qb // 2][:, (qb % 2) * 512:(qb % 2 + 1) * 512],
                                 start=(kb == 0), stop=(kb == NB16 - 1))

        pend = []
        for kb in range(NB16):
            ksl = kt_sb[half:half + 64, fb, kb * 128:(kb + 1) * 128]
            pts = []
            for t in range(2):
                sps = att_ps.tile([128, 1024], f32, name="sps")
                for g in range(2):
                    qb = 2 * t + g
                    nc.tensor.matmul(sps[:, g * 512:(g + 1) * 512], ksl,
                                     qt_sb[half:half + 64, fb, qb * 512:(qb + 1) * 512],
                                     start=True, stop=True)
                pt = ptpool.tile([128, 1024], f16, name="pt")
                if _dve_exp_tile(h, kb, t):
                    nc.vector._custom_dve(EXP8_ANT, out=pt, in0=sps,
                                          s0=t8_sb[:, h, kb:kb + 1],
                                          s1=EXP_C1, imm2=EXP_C2)
                    for _ in range(3):
                        nc.vector.tensor_mul(pt, pt, pt)
                else:
                    nc.scalar.activation(pt, sps, AF.Exp,
                                         bias=t_sb[:, h, kb:kb + 1], scale=SCALE)
                pts.append(pt)
            pend.append((kb, pts))
            if len(pend) > PIPE:
                emit_pv(*pend.pop(0))
        for item in pend:
            emit_pv(*item)
        for qb in range(4):
            rs_row = rspool.tile([1, 512], f32, name="rsrow")
            nc.vector.reciprocal(rs_row, opss[qb][DK:DK + 1, :])
            rs_bc = rspool.tile([64, 512], f32, name="rsbc")
            nc.gpsimd.partition_broadcast(rs_bc, rs_row)
            nc.vector.tensor_mul(
                a_sb[half:half + 64, fb, qb * 512:(qb + 1) * 512],
                opss[qb][0:DK, :], rs_bc)

    # ================= phase C: output projection =================
    attn.close()
    op_ps = ctx.enter_context(tc.tile_pool(name="oprj", bufs=2, space="PSUM"))
    outpool = ctx.enter_context(tc.tile_pool(name="outsb", bufs=3))
    for mb in range(NB16):
        pss = [op_ps.tile([128, 384], f32, name=f"op{j}") for j in range(2)]
        for kt3 in range(FB):
            asl = a_sb[:, kt3, mb * 128:(mb + 1) * 128]
            for j in range(2):
                nc.tensor.matmul(pss[j], asl, wo_sb[:, kt3, j * 384:(j + 1) * 384],
                                 start=(kt3 == 0), stop=(kt3 == FB - 1))
        osb = outpool.tile([128, D], f16)
        nc.vector.tensor_copy(osb[:, 0:384], pss[0])
        nc.scalar.activation(osb[:, 384:768], pss[1], AF.Copy)
        nc.sync.dma_start(out=out[mb * 128:(mb + 1) * 128, :], in_=osb)


def _build():
    nc = bacc.Bacc("TRN2", target_bir_lowering=False, debug=False, num_devices=8)
    dt = mybir.dt

    def din(name, shape, dtype):
        return nc.dram_tensor(name, list(shape), dtype, kind="ExternalInput").ap()

    io = {
        "xqT": din("xqT", (KT, 128, NSEQ), dt.float16),
        "xkvT": din("xkvT", (KT, 128, NSEQ), dt.float16),
        "wq": din("wq", (KT, 128, F), dt.float16),
        "wk": din("wk", (KT, 128, F), dt.float16),
        "wv": din("wv", (KT, 128, F), dt.float16),
        "wo": din("wo", (FB, 128, D), dt.float16),
        "cnq": din("cnq", (1, FB, 128), dt.float16),
        "cnk": din("cnk", (1, FB, 128), dt.float16),
        "cbv": din("cbv", (2, F), dt.float16),
        "bqc": din("bqc", (128, FB), dt.float16),
        "out": nc.dram_tensor("out", [NSEQ, D], dt.float16, kind="ExternalOutput").ap(),
    }

    with tile.TileContext(nc) as tc:
        with ExitStack() as ctx:
            _emit(ctx, tc, io)
    nc.compile()
    return nc


_CACHE = {}


def _get_nc():
    if "nc" not in _CACHE:
        _CACHE["nc"] = _build()
    return _CACHE["nc"]


def _prep(inputs):
    g = lambda k: np.asarray(inputs[k], dtype=np.float32)
    text, vision = g("text"), g("vision")
    ln1_w, ln1_b, ln2_w, ln2_b = g("ln1_w"), g("ln1_b"), g("ln2_w"), g("ln2_b")
    W = {nm: g("W" + nm) for nm in ("q1", "k1", "v1", "q2", "k2", "v2", "o1", "o2")}
    B = {nm: g("b" + nm) for nm in ("q1", "k1", "v1", "q2", "k2", "v2", "o1", "o2")}

    maps = [None] * 8
    for b in (0, 1):
        for path in (0, 1):
            if path == 0:
                xq, xkv = text[b], vision[b]
                lnqw, lnqb, lnkw, lnkb = ln1_w, ln1_b, ln2_w, ln2_b
                Wq, bq, Wk, Wv, bv, Wo = W["q1"], B["q1"], W["k2"], W["v2"], B["v2"], W["o1"]
            else:
                xq, xkv = vision[b], text[b]
                lnqw, lnqb, lnkw, lnkb = ln2_w, ln2_b, ln1_w, ln1_b
                Wq, bq, Wk, Wv, bv, Wo = W["q2"], B["q2"], W["k1"], W["v1"], B["v1"], W["o2"]
            xqT = np.ascontiguousarray(xq.T).astype(F16).reshape(KT, 128, NSEQ)
            xkvT = np.ascontiguousarray(xkv.T).astype(F16).reshape(KT, 128, NSEQ)
            for s in (0, 1):
                rows = slice(s * F, (s + 1) * F)
                WqT = np.ascontiguousarray(lnqw[:, None] * Wq[rows].T)
                WkT = np.ascontiguousarray(lnkw[:, None] * Wk[rows].T)
                WvT = np.ascontiguousarray(lnkw[:, None] * Wv[rows].T)
                cq = -WqT.astype(np.float32).sum(0)   # [F]
                ck = -WkT.astype(np.float32).sum(0)
                cv = -WvT.astype(np.float32).sum(0)
                bq_eff = (bq[rows] + lnqb @ Wq[rows].T).astype(np.float32)
                bv_eff = (bv[rows] + lnkb @ Wv[rows].T).astype(np.float32)
                WoT = np.ascontiguousarray(Wo[:, rows].T)  # [F, D]
                maps[b * 4 + path * 2 + s] = {
                    "xqT": xqT, "xkvT": xkvT,
                    "wq": WqT.astype(F16).reshape(KT, 128, F),
                    "wk": WkT.astype(F16).reshape(KT, 128, F),
                    "wv": WvT.astype(F16).reshape(KT, 128, F),
                    "wo": WoT.astype(F16).reshape(FB, 128, D),
                    "cnq": cq.reshape(1, FB, 128).astype(F16),
                    "cnk": ck.reshape(1, FB, 128).astype(F16),
                    "cbv": np.stack([cv, bv_eff]).astype(F16),
                    "bqc": np.ascontiguousarray(
                        bq_eff.reshape(FB, 128).T).astype(F16),
                }
    meta = (B["o1"], B["o2"])
    return maps, meta


def _unshard(results, meta):
    bo1, bo2 = meta
    text_out = np.empty((2, NSEQ, D), np.float32)
    vision_out = np.empty((2, NSEQ, D), np.float32)
    for b in (0, 1):
        text_out[b] = (results[b * 4 + 0]["out"].astype(np.float32)
                       + results[b * 4 + 1]["out"].astype(np.float32) + bo1)
        vision_out[b] = (results[b * 4 + 2]["out"].astype(np.float32)
                         + results[b * 4 + 3]["out"].astype(np.float32) + bo2)
    return (text_out, vision_out)


def run_raw(inputs, **kw):
    """Run and return the BassKernelResults (for profiling from test.py)."""
    nc = _get_nc()
    in_maps, meta = _prep(inputs)
    res = run_bass_kernel_spmd(nc, in_maps, core_ids=list(range(8)), **kw)
    return res, meta


def kernel(**inputs):
    res, meta = run_raw(inputs)
    return _unshard(res.results, meta)
